# revision 15
# baseline (speedup 1.0000x reference)
"""Multi-head attention (B=4, S=2048, D=1024, H=16) on 8 TRN2 NeuronCores.

Sharding: core c = (batch b = c//2, head-half j = c%2).  Each core computes
attention for 8 heads of one batch plus its partial output projection
(row-parallel W_o); the host sums the two partials per batch (the
"all-reduce") and stacks batches.

Per-core device algorithm (all matmuls in float32r = full-rate TF32-like):
  QT[d,s] = (W_q_shard @ x^T)    via lhsT=wqT chunks, rhs=xT chunks
  KT[d,s] likewise; V[s,dv] via lhsT=xT chunks, rhs=wvT
  per head pair / q-block / k-tile:
    S_T[k,q] = KT_h^T-slice @ QT_h  (row-packed: 2 heads of K=64 share the PE)
    P_T = exp(S_T/8)               (ACT, PSUM->SBUF)
    aug[dv+1,q] += [V|1]^T @ P_T   (row 64 = softmax denominators)
  normalize: recip (DVE) -> PE rank-2 outer-product broadcast -> DVE mul
  out[q,e] partial = cat^T-slices @ woT  -> DMA out
"""

import sys

sys.path.insert(0, "/opt/trn_rl_repo")

import numpy as np

S = 2048          # sequence length
C = 1024          # model dim (contraction for projections)
DV = 512          # per-core head dims (8 heads x 64)
HL = 8            # local heads
DK = 64
NSB = 4           # s-blocks of 512
NCT = 8           # c-tiles of 128
NDT = 4           # d-tiles of 128 (= head pairs)
NKT = 16          # k-tiles of 128
NQB = 4           # q-blocks of 512
NQT = 16          # q-tiles of 128
NEB = 2           # e-blocks of 512 (output model dim 1024)


def build_bass(debug=False):
    import concourse.bass as bass
    import concourse.mybir as mybir
    from concourse.tile import TileContext

    f32 = mybir.dt.float32
    f32r = mybir.dt.float32r
    EXP = mybir.ActivationFunctionType.Exp

    nc = bass.Bass(target_bir_lowering=False, debug=debug)

    xT = nc.declare_dram_parameter("xT", [C, S], f32, isOutput=False)
    wqT = nc.declare_dram_parameter("wqT", [C, DV], f32, isOutput=False)
    wkT = nc.declare_dram_parameter("wkT", [C, DV], f32, isOutput=False)
    wvT = nc.declare_dram_parameter("wvT", [C, DV], f32, isOutput=False)
    woT = nc.declare_dram_parameter("woT", [DV, C], f32, isOutput=False)
    onesd = nc.declare_dram_parameter("ones128", [128, 64], f32, isOutput=False)
    indd = nc.declare_dram_parameter("ind2", [2, 128], f32, isOutput=False)
    out = nc.declare_dram_parameter("out", [S, C], f32, isOutput=True)

    with nc.allow_low_precision(reason="f32r (tf32-like) matmul inputs; tol 2e-2"), TileContext(nc) as tc:
        with (
            tc.tile_pool(name="xt", bufs=10) as p_xt,
            tc.tile_pool(name="w", bufs=25) as p_w,
            tc.tile_pool(name="qk", bufs=5) as p_qk,
            tc.tile_pool(name="v", bufs=16) as p_v,
            tc.tile_pool(name="pt", bufs=4) as p_pt,
            tc.tile_pool(name="cat", bufs=4) as p_cat,
            tc.tile_pool(name="outst", bufs=2) as p_out,
            tc.tile_pool(name="misc", bufs=1) as p_misc,
            tc.tile_pool(name="recip", bufs=2) as p_recip,
            tc.tile_pool(name="psmm", bufs=2, space="PSUM") as ps_mm,
            tc.tile_pool(name="psaug", bufs=2, space="PSUM") as ps_aug,
            tc.tile_pool(name="psbc", bufs=1, space="PSUM") as ps_bc,
        ):
            # --- constants ---
            ind2 = p_misc.tile([2, 128], f32r, tag="ind2", name="ind2")
            nc.sync.dma_start(out=ind2[:, :], in_=indd[:, :].bitcast(f32r))

            # --- weights (resident) ---
            wq = []
            wk = []
            wv = []
            for ct in range(NCT):
                tq = p_w.tile([128, DV], f32r, tag="w", name=f"wq{ct}")
                nc.sync.dma_start(out=tq[:, :], in_=wqT[128 * ct : 128 * (ct + 1), :].bitcast(f32r))
                tk = p_w.tile([128, DV], f32r, tag="w", name=f"wk{ct}")
                nc.sync.dma_start(out=tk[:, :], in_=wkT[128 * ct : 128 * (ct + 1), :].bitcast(f32r))
                tv = p_w.tile([128, DV], f32r, tag="w", name=f"wv{ct}")
                nc.sync.dma_start(out=tv[:, :], in_=wvT[128 * ct : 128 * (ct + 1), :].bitcast(f32r))
                wq.append(tq)
                wk.append(tk)
                wv.append(tv)

            QT = [None] * NDT
            KT = [None] * NDT
            V = [None] * NKT
            cat = [None] * NDT

            def load_xchunks(sb):
                xc = []
                for ct in range(NCT):
                    t = p_xt.tile([128, 512], f32r, tag="xt", name=f"x{sb}_{ct}")
                    nc.sync.dma_start(
                        out=t[:, :],
                        in_=xT[128 * ct : 128 * (ct + 1), 512 * sb : 512 * (sb + 1)].bitcast(f32r),
                    )
                    xc.append(t)
                return xc

            def qk_chain(dt, sb, xc):
                """Q and K projection chains for d-tile dt over s-block sb."""
                if QT[dt] is None:
                    QT[dt] = p_qk.tile([128, S], f32r, tag="qk", name=f"qt{dt}")
                    KT[dt] = p_qk.tile([128, S], f32r, tag="qk", name=f"kt{dt}")
                ps = ps_mm.tile([128, 1024], f32, tag="mm", name=f"psqk{dt}_{sb}")
                dsl = slice(128 * dt, 128 * (dt + 1))
                for ct in range(NCT):
                    nc.tensor.matmul(
                        ps[:, 0:512],
                        lhsT=wq[ct][:, dsl],
                        rhs=xc[ct][:, :],
                        start=(ct == 0),
                        stop=(ct == NCT - 1),
                    )
                for ct in range(NCT):
                    nc.tensor.matmul(
                        ps[:, 512:1024],
                        lhsT=wk[ct][:, dsl],
                        rhs=xc[ct][:, :],
                        start=(ct == 0),
                        stop=(ct == NCT - 1),
                    )
                ssl = slice(512 * sb, 512 * (sb + 1))
                nc.vector.tensor_copy(out=QT[dt][:, ssl], in_=ps[:, 0:512])
                nc.vector.tensor_copy(out=KT[dt][:, ssl], in_=ps[:, 512:1024])

            def v_chains(sb, xc):
                """V projection for the 4 s-tiles of s-block sb."""
                for half in range(2):
                    ps = ps_mm.tile([128, 1024], f32, tag="mm", name=f"psv{sb}_{half}")
                    for loc in range(2):
                        stl = 2 * half + loc
                        for ct in range(NCT):
                            nc.tensor.matmul(
                                ps[:, 512 * loc : 512 * (loc + 1)],
                                lhsT=xc[ct][:, 128 * stl : 128 * (stl + 1)],
                                rhs=wv[ct][:, :],
                                start=(ct == 0),
                                stop=(ct == NCT - 1),
                            )
                    for loc in range(2):
                        st = 4 * sb + 2 * half + loc
                        vt = p_v.tile([128, HL * 65], f32r, tag="v", name=f"v{st}")
                        V[st] = vt
                        # ones in column 64 of each head's 65-wide strip
                        nc.sync.dma_start(
                            out=vt[:, :].rearrange("p (h x) -> p h x", x=65)[:, :, 64:65],
                            in_=onesd[:, 0:HL].bitcast(f32r),
                        )
                        nc.vector.tensor_copy(
                            out=vt[:, :].rearrange("p (h x) -> p h x", x=65)[:, :, 0:64],
                            in_=ps[:, 512 * loc : 512 * (loc + 1)].rearrange(
                                "p (h x) -> p h x", x=64
                            ),
                        )

            # --- phase 1: V + Q/K for d-tile 0 ---
            for sb in range(NSB):
                xc = load_xchunks(sb)
                v_chains(sb, xc)
                qk_chain(0, sb, xc)

            # deferred QK-projection tasks for later head pairs, interleaved
            # into the previous pair's attention to keep ACT fed
            def make_qk_tasks(dt):
                def task(sb=None, dt=dt):
                    xc = load_xchunks(sb)
                    qk_chain(dt, sb, xc)

                return [lambda sb=sb: task(sb) for sb in range(NSB)]

            pending = {j: make_qk_tasks(j) for j in range(1, NDT)}

            # deferred W_o chains (by q-block), interleaved into pair-3
            wo_tiles = {}

            def load_wo():
                for jj in range(NDT):
                    for eb in range(NEB):
                        t = p_w.tile([128, 512], f32r, tag="w", name=f"wo{jj}_{eb}")
                        nc.sync.dma_start(
                            out=t[:, :],
                            in_=woT[
                                128 * jj : 128 * (jj + 1), 512 * eb : 512 * (eb + 1)
                            ].bitcast(f32r),
                        )
                        wo_tiles[(jj, eb)] = t

            def wo_chains(qb):
                """Output projection for the 4 q-tiles of q-block qb."""
                for qtl in range(4):
                    qt = 4 * qb + qtl
                    qsl = slice(128 * qt, 128 * (qt + 1))
                    for eb in range(NEB):
                        ps = ps_mm.tile([128, 512], f32, tag="mm", name=f"pso{qt}_{eb}")
                        for jj in range(NDT):
                            nc.tensor.matmul(
                                ps[:, :],
                                lhsT=cat[jj][:, qsl],
                                rhs=wo_tiles[(jj, eb)][:, :],
                                start=(jj == 0),
                                stop=(jj == NDT - 1),
                            )
                        ost = p_out.tile([128, 512], f32, tag="outst", name=f"o{qt}_{eb}")
                        nc.vector.tensor_copy(out=ost[:, :], in_=ps[:, :])
                        nc.sync.dma_start(
                            out=out[qsl, 512 * eb : 512 * (eb + 1)], in_=ost[:, :]
                        )

            # --- phase 2: attention per head pair ---
            for j in range(NDT):
                hA, hB = 2 * j, 2 * j + 1
                qtj, ktj = QT[j], KT[j]
                interleave = pending.pop(j + 1, [])
                if j == NDT - 1:
                    load_wo()
                cat[j] = p_cat.tile([128, S], f32r, tag="cat", name=f"cat{j}")
                for qb in range(NQB):
                    qsl = slice(512 * qb, 512 * (qb + 1))
                    augA = ps_aug.tile([65, 512], f32, tag="aug", name=f"augA{j}_{qb}")
                    augB = ps_aug.tile([65, 512], f32, tag="aug", name=f"augB{j}_{qb}")
                    for g in range(8):
                        psA = ps_mm.tile([128, 1024], f32, tag="mm", name=f"psA{j}{qb}{g}")
                        psB = ps_mm.tile([128, 1024], f32, tag="mm", name=f"psB{j}{qb}{g}")
                        for i in range(2):
                            kt = 2 * g + i
                            ksl = slice(128 * kt, 128 * (kt + 1))
                            osl = slice(512 * i, 512 * (i + 1))
                            nc.tensor.matmul(
                                psA[:, osl],
                                lhsT=ktj[0:64, ksl],
                                rhs=qtj[0:64, qsl],
                                start=True,
                                stop=True,
                            )
                            nc.tensor.matmul(
                                psB[:, osl],
                                lhsT=ktj[64:128, ksl],
                                rhs=qtj[64:128, qsl],
                                start=True,
                                stop=True,
                            )
                        ptA = p_pt.tile([128, 1024], f32r, tag="pt", name=f"ptA{j}{qb}{g}")
                        ptB = p_pt.tile([128, 1024], f32r, tag="pt", name=f"ptB{j}{qb}{g}")
                        nc.scalar.activation(out=ptA[:, :], in_=psA[:, :], func=EXP, scale=0.125)
                        nc.scalar.activation(out=ptB[:, :], in_=psB[:, :], func=EXP, scale=0.125)
                        for i in range(2):
                            kt = 2 * g + i
                            nc.tensor.matmul(
                                augA[:, :],
                                lhsT=V[kt][:, 65 * hA : 65 * hA + 65],
                                rhs=ptA[:, 512 * i : 512 * (i + 1)],
                                start=(kt == 0),
                                stop=(kt == NKT - 1),
                            )
                            nc.tensor.matmul(
                                augB[:, :],
                                lhsT=V[kt][:, 65 * hB : 65 * hB + 65],
                                rhs=ptB[:, 512 * i : 512 * (i + 1)],
                                start=(kt == 0),
                                stop=(kt == NKT - 1),
                            )
                        # keep PE fed with next pair's projections during ACT-bound stretch
                        if g % 2 == 1 and interleave:
                            interleave.pop(0)()
                    # normalize: recip -> gather to [2,512] -> rank-2 outer product
                    rtA = p_recip.tile([1, 512], f32r, tag="rt", name=f"rtA{j}_{qb}")
                    rtB = p_recip.tile([1, 512], f32r, tag="rt", name=f"rtB{j}_{qb}")
                    nc.vector.reciprocal(out=rtA[:, :], in_=augA[64:65, :])
                    nc.vector.reciprocal(out=rtB[:, :], in_=augB[64:65, :])
                    rt2 = p_recip.tile([2, 512], f32r, tag="rt2", name=f"rt2{j}_{qb}")
                    nc.sync.dma_start(out=rt2[0:1, :], in_=rtA[:, :])
                    nc.sync.dma_start(out=rt2[1:2, :], in_=rtB[:, :])
                    bc = ps_bc.tile([128, 512], f32, tag="bc", name=f"bc{j}_{qb}")
                    nc.tensor.matmul(
                        bc[:, :],
                        lhsT=ind2[:, :],
                        rhs=rt2[:, :],
                        start=True,
                        stop=True,
                    )
                    nc.vector.tensor_copy(out=cat[j][0:64, qsl], in_=augA[0:64, :])
                    nc.vector.tensor_copy(out=cat[j][64:128, qsl], in_=augB[0:64, :])
                    nc.vector.tensor_mul(
                        out=cat[j][0:64, qsl], in0=cat[j][0:64, qsl], in1=bc[0:64, :]
                    )
                    nc.vector.tensor_mul(
                        out=cat[j][64:128, qsl], in0=cat[j][64:128, qsl], in1=bc[64:128, :]
                    )
                    if j == NDT - 1:
                        wo_chains(qb)
                # anything not interleaved (shouldn't happen, but be safe)
                for t in interleave:
                    t()

    _split_matmul_waits(nc)
    return nc


_SPLIT_TYPES = {"InstMatmult", "InstDMACopy", "InstActivation", "InstTensorCopy", "InstTensorTensor", "InstMemSet", "InstTensorScalarPtr", "InstTensorReduce", "InstReciprocal", "InstDrain", "InstNoOp", "InstEventSemaphore"}


def _split_matmul_waits(nc):
    """Several walrus instruction structs (fused-weight-load matmul S3_LW,
    DMA_DIRECT2D, ...) accept only one sync wait.  Move extra waits onto
    standalone no-ops on the same engine placed just before the instruction."""
    import concourse.mybir as mybir

    noop_cls = None
    for f in nc.m.functions:
        for blk in f.blocks:
            patched = []
            for inst in blk.instructions:
                si = getattr(inst, "sync_info", None)
                if (
                    type(inst).__name__ in _SPLIT_TYPES
                    and si is not None
                    and si.on_wait
                    and len(si.on_wait) > 1
                ):
                    if noop_cls is None:
                        import bass_rust

                        noop_cls = bass_rust.InstNoOp
                    waits = list(si.on_wait)
                    for w in waits[:-1]:
                        nop = noop_cls(
                            name=f"I-wsplit-{nc.next_id()}",
                            engine=inst.engine,
                            ins=[],
                            outs=[],
                        )
                        nop.sync_info = mybir.SyncInfo(on_wait=[w], on_update=[])
                        patched.append(nop)
                    inst.sync_info = mybir.SyncInfo(
                        on_wait=[waits[-1]], on_update=si.on_update
                    )
                patched.append(inst)
            blk.instructions[:] = patched


_CACHE = {}

_IND2 = np.zeros((2, 128), dtype=np.float32)
_IND2[0, 0:64] = 1.0
_IND2[1, 64:128] = 1.0


def kernel(x, W_q, W_k, W_v, W_o):
    x = np.asarray(x, dtype=np.float32)
    W_q = np.asarray(W_q, dtype=np.float32)
    W_k = np.asarray(W_k, dtype=np.float32)
    W_v = np.asarray(W_v, dtype=np.float32)
    W_o = np.asarray(W_o, dtype=np.float32)

    if "nc" not in _CACHE:
        _CACHE["nc"] = build_bass()
    nc = _CACHE["nc"]

    from concourse.bass_utils import run_bass_kernel_spmd

    in_maps = []
    for c in range(8):
        b, j = divmod(c, 2)
        sl = slice(512 * j, 512 * (j + 1))
        in_maps.append(
            {
                "xT": np.ascontiguousarray(x[b].T),
                "wqT": np.ascontiguousarray(W_q[sl, :].T),
                "wkT": np.ascontiguousarray(W_k[sl, :].T),
                "wvT": np.ascontiguousarray(W_v[sl, :].T),
                "woT": np.ascontiguousarray(W_o[:, sl].T),
                "ones128": np.ones((128, 64), dtype=np.float32),
                "ind2": _IND2,
            }
        )

    res = run_bass_kernel_spmd(nc, in_maps, list(range(8))).results
    out = np.empty((4, S, C), dtype=np.float32)
    for b in range(4):
        out[b] = res[2 * b]["out"] + res[2 * b + 1]["out"]
    return out


# revision 16
# speedup vs baseline: 1.0749x; 1.0749x over previous
"""Multi-head attention (B=4, S=2048, D=1024, H=16) on 8 TRN2 NeuronCores.

Sharding: core c = (batch b = c//2, head-half j = c%2).  Each core computes
attention for 8 heads of one batch plus its partial output projection
(row-parallel W_o); the host sums the two partials per batch (the
"all-reduce") and stacks batches.

Per-core device algorithm (all matmuls in float32r = full-rate TF32-like):
  QT[d,s] = (W_q_shard @ x^T)    via lhsT=wqT chunks, rhs=xT chunks
  KT[d,s] likewise; V[s,dv] via lhsT=xT chunks, rhs=wvT
  per head pair / q-block / k-tile:
    S_T[k,q] = KT_h^T-slice @ QT_h  (row-packed: 2 heads of K=64 share the PE)
    P_T = exp(S_T/8)               (ACT, PSUM->SBUF)
    aug[dv+1,q] += [V|1]^T @ P_T   (row 64 = softmax denominators)
  normalize: recip (DVE) -> PE rank-2 outer-product broadcast -> DVE mul
  out[q,e] partial = cat^T-slices @ woT  -> DMA out
"""

import sys

sys.path.insert(0, "/opt/trn_rl_repo")

import numpy as np

S = 2048          # sequence length
C = 1024          # model dim (contraction for projections)
DV = 512          # per-core head dims (8 heads x 64)
HL = 8            # local heads
DK = 64
NSB = 4           # s-blocks of 512
NCT = 8           # c-tiles of 128
NDT = 4           # d-tiles of 128 (= head pairs)
NKT = 16          # k-tiles of 128
NQB = 4           # q-blocks of 512
NQT = 16          # q-tiles of 128
NEB = 2           # e-blocks of 512 (output model dim 1024)


def build_bass(debug=False):
    import concourse.bass as bass
    import concourse.mybir as mybir
    from concourse.tile import TileContext

    f32 = mybir.dt.float32
    f32r = mybir.dt.float32r
    bf16 = mybir.dt.bfloat16
    EXP = mybir.ActivationFunctionType.Exp

    nc = bass.Bass(target_bir_lowering=False, debug=debug)

    xT = nc.declare_dram_parameter("xT", [C, S], f32, isOutput=False)
    wqT = nc.declare_dram_parameter("wqT", [C, DV], f32, isOutput=False)
    wkT = nc.declare_dram_parameter("wkT", [C, DV], f32, isOutput=False)
    wvT = nc.declare_dram_parameter("wvT", [C, DV], f32, isOutput=False)
    woT = nc.declare_dram_parameter("woT", [DV, C], f32, isOutput=False)
    onesd = nc.declare_dram_parameter("ones128", [128, 64], mybir.dt.bfloat16, isOutput=False)
    indd = nc.declare_dram_parameter("ind2", [2, 128], f32, isOutput=False)
    out = nc.declare_dram_parameter("out", [S, C], f32, isOutput=True)

    with nc.allow_low_precision(reason="f32r (tf32-like) matmul inputs; tol 2e-2"), TileContext(nc) as tc:
        with (
            tc.tile_pool(name="xt", bufs=10) as p_xt,
            tc.tile_pool(name="w", bufs=25) as p_w,
            tc.tile_pool(name="qk", bufs=5) as p_qk,
            tc.tile_pool(name="v", bufs=16) as p_v,
            tc.tile_pool(name="pt", bufs=4) as p_pt,
            tc.tile_pool(name="cat", bufs=4) as p_cat,
            tc.tile_pool(name="outst", bufs=2) as p_out,
            tc.tile_pool(name="misc", bufs=1) as p_misc,
            tc.tile_pool(name="recip", bufs=2) as p_recip,
            tc.tile_pool(name="psmm", bufs=2, space="PSUM") as ps_mm,
            tc.tile_pool(name="psaug", bufs=2, space="PSUM") as ps_aug,
            tc.tile_pool(name="psbc", bufs=1, space="PSUM") as ps_bc,
        ):
            # --- constants ---
            ind2 = p_misc.tile([2, 128], f32r, tag="ind2", name="ind2")
            nc.sync.dma_start(out=ind2[:, :], in_=indd[:, :].bitcast(f32r))

            # --- weights (resident) ---
            wq = []
            wk = []
            wv = []
            for ct in range(NCT):
                tq = p_w.tile([128, DV], f32r, tag="w", name=f"wq{ct}")
                nc.sync.dma_start(out=tq[:, :], in_=wqT[128 * ct : 128 * (ct + 1), :].bitcast(f32r))
                tk = p_w.tile([128, DV], f32r, tag="w", name=f"wk{ct}")
                nc.sync.dma_start(out=tk[:, :], in_=wkT[128 * ct : 128 * (ct + 1), :].bitcast(f32r))
                tv = p_w.tile([128, DV], f32r, tag="w", name=f"wv{ct}")
                nc.sync.dma_start(out=tv[:, :], in_=wvT[128 * ct : 128 * (ct + 1), :].bitcast(f32r))
                wq.append(tq)
                wk.append(tk)
                wv.append(tv)

            QT = [None] * NDT
            KT = [None] * NDT
            V = [None] * NKT
            cat = [None] * NDT

            def load_xchunks(sb):
                xc = []
                for ct in range(NCT):
                    t = p_xt.tile([128, 512], f32r, tag="xt", name=f"x{sb}_{ct}")
                    nc.sync.dma_start(
                        out=t[:, :],
                        in_=xT[128 * ct : 128 * (ct + 1), 512 * sb : 512 * (sb + 1)].bitcast(f32r),
                    )
                    xc.append(t)
                return xc

            def qk_chain(dt, sb, xc):
                """Q and K projection chains for d-tile dt over s-block sb."""
                if QT[dt] is None:
                    QT[dt] = p_qk.tile([128, S], bf16, tag="qk", name=f"qt{dt}")
                    KT[dt] = p_qk.tile([128, S], bf16, tag="qk", name=f"kt{dt}")
                ps = ps_mm.tile([128, 1024], f32, tag="mm", name=f"psqk{dt}_{sb}")
                dsl = slice(128 * dt, 128 * (dt + 1))
                for ct in range(NCT):
                    nc.tensor.matmul(
                        ps[:, 0:512],
                        lhsT=wq[ct][:, dsl],
                        rhs=xc[ct][:, :],
                        start=(ct == 0),
                        stop=(ct == NCT - 1),
                    )
                for ct in range(NCT):
                    nc.tensor.matmul(
                        ps[:, 512:1024],
                        lhsT=wk[ct][:, dsl],
                        rhs=xc[ct][:, :],
                        start=(ct == 0),
                        stop=(ct == NCT - 1),
                    )
                ssl = slice(512 * sb, 512 * (sb + 1))
                nc.vector.tensor_copy(out=QT[dt][:, ssl], in_=ps[:, 0:512])
                nc.vector.tensor_copy(out=KT[dt][:, ssl], in_=ps[:, 512:1024])

            def v_chains(sb, xc):
                """V projection for the 4 s-tiles of s-block sb."""
                for half in range(2):
                    ps = ps_mm.tile([128, 1024], f32, tag="mm", name=f"psv{sb}_{half}")
                    for loc in range(2):
                        stl = 2 * half + loc
                        for ct in range(NCT):
                            nc.tensor.matmul(
                                ps[:, 512 * loc : 512 * (loc + 1)],
                                lhsT=xc[ct][:, 128 * stl : 128 * (stl + 1)],
                                rhs=wv[ct][:, :],
                                start=(ct == 0),
                                stop=(ct == NCT - 1),
                            )
                    for loc in range(2):
                        st = 4 * sb + 2 * half + loc
                        vt = p_v.tile([128, HL * 65], bf16, tag="v", name=f"v{st}")
                        V[st] = vt
                        # ones in column 64 of each head's 65-wide strip
                        nc.sync.dma_start(
                            out=vt[:, :].rearrange("p (h x) -> p h x", x=65)[:, :, 64:65],
                            in_=onesd[:, 0:HL],
                        )
                        nc.vector.tensor_copy(
                            out=vt[:, :].rearrange("p (h x) -> p h x", x=65)[:, :, 0:64],
                            in_=ps[:, 512 * loc : 512 * (loc + 1)].rearrange(
                                "p (h x) -> p h x", x=64
                            ),
                        )

            # --- phase 1: V + Q/K for d-tile 0 ---
            for sb in range(NSB):
                xc = load_xchunks(sb)
                v_chains(sb, xc)
                qk_chain(0, sb, xc)

            # deferred QK-projection tasks for later head pairs, interleaved
            # into the previous pair's attention to keep ACT fed
            def make_qk_tasks(dt):
                def task(sb=None, dt=dt):
                    xc = load_xchunks(sb)
                    qk_chain(dt, sb, xc)

                return [lambda sb=sb: task(sb) for sb in range(NSB)]

            pending = {j: make_qk_tasks(j) for j in range(1, NDT)}

            # deferred W_o chains (by q-block), interleaved into pair-3
            wo_tiles = {}

            def load_wo():
                for jj in range(NDT):
                    for eb in range(NEB):
                        t = p_w.tile([128, 512], f32r, tag="w", name=f"wo{jj}_{eb}")
                        nc.sync.dma_start(
                            out=t[:, :],
                            in_=woT[
                                128 * jj : 128 * (jj + 1), 512 * eb : 512 * (eb + 1)
                            ].bitcast(f32r),
                        )
                        wo_tiles[(jj, eb)] = t

            def wo_chains(qb):
                """Output projection for the 4 q-tiles of q-block qb."""
                for qtl in range(4):
                    qt = 4 * qb + qtl
                    qsl = slice(128 * qt, 128 * (qt + 1))
                    for eb in range(NEB):
                        ps = ps_mm.tile([128, 512], f32, tag="mm", name=f"pso{qt}_{eb}")
                        for jj in range(NDT):
                            nc.tensor.matmul(
                                ps[:, :],
                                lhsT=cat[jj][:, qsl],
                                rhs=wo_tiles[(jj, eb)][:, :],
                                start=(jj == 0),
                                stop=(jj == NDT - 1),
                            )
                        ost = p_out.tile([128, 512], f32, tag="outst", name=f"o{qt}_{eb}")
                        nc.vector.tensor_copy(out=ost[:, :], in_=ps[:, :])
                        nc.sync.dma_start(
                            out=out[qsl, 512 * eb : 512 * (eb + 1)], in_=ost[:, :]
                        )

            # --- phase 2: attention per head pair ---
            for j in range(NDT):
                hA, hB = 2 * j, 2 * j + 1
                qtj, ktj = QT[j], KT[j]
                interleave = pending.pop(j + 1, [])
                if j == NDT - 1:
                    load_wo()
                cat[j] = p_cat.tile([128, S], f32r, tag="cat", name=f"cat{j}")
                for qb in range(NQB):
                    qsl = slice(512 * qb, 512 * (qb + 1))
                    augA = ps_aug.tile([65, 512], f32, tag="aug", name=f"augA{j}_{qb}")
                    augB = ps_aug.tile([65, 512], f32, tag="aug", name=f"augB{j}_{qb}")
                    for g in range(8):
                        psA = ps_mm.tile([128, 1024], f32, tag="mm", name=f"psA{j}{qb}{g}")
                        psB = ps_mm.tile([128, 1024], f32, tag="mm", name=f"psB{j}{qb}{g}")
                        for i in range(2):
                            kt = 2 * g + i
                            ksl = slice(128 * kt, 128 * (kt + 1))
                            osl = slice(512 * i, 512 * (i + 1))
                            nc.tensor.matmul(
                                psA[:, osl],
                                lhsT=ktj[0:64, ksl],
                                rhs=qtj[0:64, qsl],
                                start=True,
                                stop=True,
                            )
                            nc.tensor.matmul(
                                psB[:, osl],
                                lhsT=ktj[64:128, ksl],
                                rhs=qtj[64:128, qsl],
                                start=True,
                                stop=True,
                            )
                        ptA = p_pt.tile([128, 1024], bf16, tag="pt", name=f"ptA{j}{qb}{g}")
                        ptB = p_pt.tile([128, 1024], bf16, tag="pt", name=f"ptB{j}{qb}{g}")
                        nc.scalar.activation(out=ptA[:, :], in_=psA[:, :], func=EXP, scale=0.125)
                        nc.scalar.activation(out=ptB[:, :], in_=psB[:, :], func=EXP, scale=0.125)
                        for i in range(2):
                            kt = 2 * g + i
                            nc.tensor.matmul(
                                augA[:, :],
                                lhsT=V[kt][:, 65 * hA : 65 * hA + 65],
                                rhs=ptA[:, 512 * i : 512 * (i + 1)],
                                start=(kt == 0),
                                stop=(kt == NKT - 1),
                            )
                            nc.tensor.matmul(
                                augB[:, :],
                                lhsT=V[kt][:, 65 * hB : 65 * hB + 65],
                                rhs=ptB[:, 512 * i : 512 * (i + 1)],
                                start=(kt == 0),
                                stop=(kt == NKT - 1),
                            )
                        # keep PE fed with next pair's projections during ACT-bound stretch
                        if g % 2 == 1 and interleave:
                            interleave.pop(0)()
                    # normalize: recip -> gather to [2,512] -> rank-2 outer product
                    rtA = p_recip.tile([1, 512], f32r, tag="rt", name=f"rtA{j}_{qb}")
                    rtB = p_recip.tile([1, 512], f32r, tag="rt", name=f"rtB{j}_{qb}")
                    nc.vector.reciprocal(out=rtA[:, :], in_=augA[64:65, :])
                    nc.vector.reciprocal(out=rtB[:, :], in_=augB[64:65, :])
                    rt2 = p_recip.tile([2, 512], f32r, tag="rt2", name=f"rt2{j}_{qb}")
                    nc.sync.dma_start(out=rt2[0:1, :], in_=rtA[:, :])
                    nc.sync.dma_start(out=rt2[1:2, :], in_=rtB[:, :])
                    bc = ps_bc.tile([128, 512], f32, tag="bc", name=f"bc{j}_{qb}")
                    nc.tensor.matmul(
                        bc[:, :],
                        lhsT=ind2[:, :],
                        rhs=rt2[:, :],
                        start=True,
                        stop=True,
                    )
                    nc.vector.tensor_copy(out=cat[j][0:64, qsl], in_=augA[0:64, :])
                    nc.vector.tensor_copy(out=cat[j][64:128, qsl], in_=augB[0:64, :])
                    nc.vector.tensor_mul(
                        out=cat[j][0:64, qsl], in0=cat[j][0:64, qsl], in1=bc[0:64, :]
                    )
                    nc.vector.tensor_mul(
                        out=cat[j][64:128, qsl], in0=cat[j][64:128, qsl], in1=bc[64:128, :]
                    )
                    if j == NDT - 1:
                        wo_chains(qb)
                # anything not interleaved (shouldn't happen, but be safe)
                for t in interleave:
                    t()

    _split_matmul_waits(nc)
    return nc


_SPLIT_TYPES = {"InstMatmult", "InstDMACopy", "InstActivation", "InstTensorCopy", "InstTensorTensor", "InstMemSet", "InstTensorScalarPtr", "InstTensorReduce", "InstReciprocal", "InstDrain", "InstNoOp", "InstEventSemaphore"}


def _split_matmul_waits(nc):
    """Several walrus instruction structs (fused-weight-load matmul S3_LW,
    DMA_DIRECT2D, ...) accept only one sync wait.  Move extra waits onto
    standalone no-ops on the same engine placed just before the instruction."""
    import concourse.mybir as mybir

    noop_cls = None
    for f in nc.m.functions:
        for blk in f.blocks:
            patched = []
            for inst in blk.instructions:
                si = getattr(inst, "sync_info", None)
                if (
                    type(inst).__name__ in _SPLIT_TYPES
                    and si is not None
                    and si.on_wait
                    and len(si.on_wait) > 1
                ):
                    if noop_cls is None:
                        import bass_rust

                        noop_cls = bass_rust.InstNoOp
                    waits = list(si.on_wait)
                    for w in waits[:-1]:
                        nop = noop_cls(
                            name=f"I-wsplit-{nc.next_id()}",
                            engine=inst.engine,
                            ins=[],
                            outs=[],
                        )
                        nop.sync_info = mybir.SyncInfo(on_wait=[w], on_update=[])
                        patched.append(nop)
                    inst.sync_info = mybir.SyncInfo(
                        on_wait=[waits[-1]], on_update=si.on_update
                    )
                patched.append(inst)
            blk.instructions[:] = patched


_CACHE = {}

import ml_dtypes

_BF16 = ml_dtypes.bfloat16

_IND2 = np.zeros((2, 128), dtype=np.float32)
_IND2[0, 0:64] = 1.0
_IND2[1, 64:128] = 1.0


def kernel(x, W_q, W_k, W_v, W_o):
    x = np.asarray(x, dtype=np.float32)
    W_q = np.asarray(W_q, dtype=np.float32)
    W_k = np.asarray(W_k, dtype=np.float32)
    W_v = np.asarray(W_v, dtype=np.float32)
    W_o = np.asarray(W_o, dtype=np.float32)

    if "nc" not in _CACHE:
        _CACHE["nc"] = build_bass()
    nc = _CACHE["nc"]

    from concourse.bass_utils import run_bass_kernel_spmd

    in_maps = []
    for c in range(8):
        b, j = divmod(c, 2)
        sl = slice(512 * j, 512 * (j + 1))
        in_maps.append(
            {
                "xT": np.ascontiguousarray(x[b].T),
                "wqT": np.ascontiguousarray(W_q[sl, :].T),
                "wkT": np.ascontiguousarray(W_k[sl, :].T),
                "wvT": np.ascontiguousarray(W_v[sl, :].T),
                "woT": np.ascontiguousarray(W_o[:, sl].T),
                "ones128": np.ones((128, 64), dtype=_BF16),
                "ind2": _IND2,
            }
        )

    res = run_bass_kernel_spmd(nc, in_maps, list(range(8))).results
    out = np.empty((4, S, C), dtype=np.float32)
    for b in range(4):
        out[b] = res[2 * b]["out"] + res[2 * b + 1]["out"]
    return out


# revision 19
# speedup vs baseline: 1.3133x; 1.2218x over previous
"""Multi-head attention (B=4, S=2048, D=1024, H=16) on 8 TRN2 NeuronCores.

Sharding: core c = (batch b = c//2, head-half j = c%2).  Each core computes
attention for 8 heads of one batch plus its partial output projection
(row-parallel W_o); the host sums the two partials per batch (the
"all-reduce") and stacks batches.

Per-core device algorithm (all matmuls in float32r = full-rate TF32-like):
  QT[d,s] = (W_q_shard @ x^T)    via lhsT=wqT chunks, rhs=xT chunks
  KT[d,s] likewise; V[s,dv] via lhsT=xT chunks, rhs=wvT
  per head pair / q-block / k-tile:
    S_T[k,q] = KT_h^T-slice @ QT_h  (row-packed: 2 heads of K=64 share the PE)
    P_T = exp(S_T/8)               (ACT, PSUM->SBUF)
    aug[dv+1,q] += [V|1]^T @ P_T   (row 64 = softmax denominators)
  normalize: recip (DVE) -> PE rank-2 outer-product broadcast -> DVE mul
  out[q,e] partial = cat^T-slices @ woT  -> DMA out
"""

import sys

sys.path.insert(0, "/opt/trn_rl_repo")

import numpy as np

S = 2048          # sequence length
C = 1024          # model dim (contraction for projections)
DV = 512          # per-core head dims (8 heads x 64)
HL = 8            # local heads
DK = 64
NSB = 4           # s-blocks of 512
NCT = 8           # c-tiles of 128
NDT = 4           # d-tiles of 128 (= head pairs)
NKT = 16          # k-tiles of 128
NQB = 4           # q-blocks of 512
NQT = 16          # q-tiles of 128
NEB = 2           # e-blocks of 512 (output model dim 1024)


def build_bass(debug=False):
    import concourse.bass as bass
    import concourse.mybir as mybir
    from concourse.tile import TileContext

    f32 = mybir.dt.float32
    f32r = mybir.dt.float32r
    bf16 = mybir.dt.bfloat16
    EXP = mybir.ActivationFunctionType.Exp

    nc = bass.Bass(target_bir_lowering=False, debug=debug)

    xT = nc.declare_dram_parameter("xT", [C, S], f32, isOutput=False)
    wqT = nc.declare_dram_parameter("wqT", [C, DV], f32, isOutput=False)
    wkT = nc.declare_dram_parameter("wkT", [C, DV], f32, isOutput=False)
    wvT = nc.declare_dram_parameter("wvT", [C, DV], f32, isOutput=False)
    woT = nc.declare_dram_parameter("woT", [DV, C], f32, isOutput=False)
    onesd = nc.declare_dram_parameter("ones128", [128, 64], mybir.dt.bfloat16, isOutput=False)
    indd = nc.declare_dram_parameter("ind2", [2, 128], f32, isOutput=False)
    out = nc.declare_dram_parameter("out", [S, C], f32, isOutput=True)

    with nc.allow_low_precision(reason="f32r (tf32-like) matmul inputs; tol 2e-2"), TileContext(nc) as tc:
        with (
            tc.tile_pool(name="xt", bufs=10) as p_xt,
            tc.tile_pool(name="w", bufs=25) as p_w,
            tc.tile_pool(name="qk", bufs=5) as p_qk,
            tc.tile_pool(name="v", bufs=16) as p_v,
            tc.tile_pool(name="pt", bufs=24) as p_pt,
            tc.tile_pool(name="cat", bufs=4) as p_cat,
            tc.tile_pool(name="outst", bufs=2) as p_out,
            tc.tile_pool(name="misc", bufs=1) as p_misc,
            tc.tile_pool(name="recip", bufs=2) as p_recip,
            tc.tile_pool(name="psmm", bufs=2, space="PSUM") as ps_mm,
            tc.tile_pool(name="psaug", bufs=4, space="PSUM") as ps_aug,
        ):
            # --- constants ---
            ind2 = p_misc.tile([2, 128], f32r, tag="ind2", name="ind2")
            nc.sync.dma_start(out=ind2[:, :], in_=indd[:, :].bitcast(f32r))

            # --- weights (resident) ---
            wq = []
            wk = []
            wv = []
            for ct in range(NCT):
                tq = p_w.tile([128, DV], f32r, tag="w", name=f"wq{ct}")
                nc.sync.dma_start(out=tq[:, :], in_=wqT[128 * ct : 128 * (ct + 1), :].bitcast(f32r))
                tk = p_w.tile([128, DV], f32r, tag="w", name=f"wk{ct}")
                nc.sync.dma_start(out=tk[:, :], in_=wkT[128 * ct : 128 * (ct + 1), :].bitcast(f32r))
                tv = p_w.tile([128, DV], f32r, tag="w", name=f"wv{ct}")
                nc.sync.dma_start(out=tv[:, :], in_=wvT[128 * ct : 128 * (ct + 1), :].bitcast(f32r))
                wq.append(tq)
                wk.append(tk)
                wv.append(tv)

            QT = [None] * NDT
            KT = [None] * NDT
            V = [None] * NKT
            cat = [None] * NDT

            def load_xchunks(sb):
                xc = []
                for ct in range(NCT):
                    t = p_xt.tile([128, 512], f32r, tag="xt", name=f"x{sb}_{ct}")
                    nc.sync.dma_start(
                        out=t[:, :],
                        in_=xT[128 * ct : 128 * (ct + 1), 512 * sb : 512 * (sb + 1)].bitcast(f32r),
                    )
                    xc.append(t)
                return xc

            def qk_chain(dt, sb, xc):
                """Q and K projection chains for d-tile dt over s-block sb."""
                if QT[dt] is None:
                    QT[dt] = p_qk.tile([128, S], bf16, tag="qk", name=f"qt{dt}")
                    KT[dt] = p_qk.tile([128, S], bf16, tag="qk", name=f"kt{dt}")
                ps = ps_mm.tile([128, 1024], f32, tag="mm", name=f"psqk{dt}_{sb}")
                dsl = slice(128 * dt, 128 * (dt + 1))
                for ct in range(NCT):
                    nc.tensor.matmul(
                        ps[:, 0:512],
                        lhsT=wq[ct][:, dsl],
                        rhs=xc[ct][:, :],
                        start=(ct == 0),
                        stop=(ct == NCT - 1),
                    )
                for ct in range(NCT):
                    nc.tensor.matmul(
                        ps[:, 512:1024],
                        lhsT=wk[ct][:, dsl],
                        rhs=xc[ct][:, :],
                        start=(ct == 0),
                        stop=(ct == NCT - 1),
                    )
                ssl = slice(512 * sb, 512 * (sb + 1))
                nc.vector.tensor_copy(out=QT[dt][:, ssl], in_=ps[:, 0:512])
                nc.vector.tensor_copy(out=KT[dt][:, ssl], in_=ps[:, 512:1024])

            def v_chains(sb, xc):
                """V projection for the 4 s-tiles of s-block sb."""
                for half in range(2):
                    ps = ps_mm.tile([128, 1024], f32, tag="mm", name=f"psv{sb}_{half}")
                    for loc in range(2):
                        stl = 2 * half + loc
                        for ct in range(NCT):
                            nc.tensor.matmul(
                                ps[:, 512 * loc : 512 * (loc + 1)],
                                lhsT=xc[ct][:, 128 * stl : 128 * (stl + 1)],
                                rhs=wv[ct][:, :],
                                start=(ct == 0),
                                stop=(ct == NCT - 1),
                            )
                    for loc in range(2):
                        st = 4 * sb + 2 * half + loc
                        vt = p_v.tile([128, HL * 65], bf16, tag="v", name=f"v{st}")
                        V[st] = vt
                        # ones in column 64 of each head's 65-wide strip
                        nc.sync.dma_start(
                            out=vt[:, :].rearrange("p (h x) -> p h x", x=65)[:, :, 64:65],
                            in_=onesd[:, 0:HL],
                        )
                        nc.vector.tensor_copy(
                            out=vt[:, :].rearrange("p (h x) -> p h x", x=65)[:, :, 0:64],
                            in_=ps[:, 512 * loc : 512 * (loc + 1)].rearrange(
                                "p (h x) -> p h x", x=64
                            ),
                        )

            # --- phase 1: V + Q/K for d-tile 0 ---
            for sb in range(NSB):
                xc = load_xchunks(sb)
                v_chains(sb, xc)
                qk_chain(0, sb, xc)

            # deferred QK-projection tasks for later head pairs, interleaved
            # into the previous pair's attention to keep ACT fed
            def make_qk_tasks(dt):
                def task(sb=None, dt=dt):
                    xc = load_xchunks(sb)
                    qk_chain(dt, sb, xc)

                return [lambda sb=sb: task(sb) for sb in range(NSB)]

            pending = {j: make_qk_tasks(j) for j in range(1, NDT)}

            # deferred W_o chains (by q-block), interleaved into pair-3
            wo_tiles = {}

            def load_wo():
                for jj in range(NDT):
                    for eb in range(NEB):
                        t = p_w.tile([128, 512], f32r, tag="w", name=f"wo{jj}_{eb}")
                        nc.sync.dma_start(
                            out=t[:, :],
                            in_=woT[
                                128 * jj : 128 * (jj + 1), 512 * eb : 512 * (eb + 1)
                            ].bitcast(f32r),
                        )
                        wo_tiles[(jj, eb)] = t

            def wo_chains(qb):
                """Output projection for the 4 q-tiles of q-block qb."""
                for qtl in range(4):
                    qt = 4 * qb + qtl
                    qsl = slice(128 * qt, 128 * (qt + 1))
                    for eb in range(NEB):
                        ps = ps_mm.tile([128, 512], f32, tag="mm", name=f"pso{qt}_{eb}")
                        for jj in range(NDT):
                            nc.tensor.matmul(
                                ps[:, :],
                                lhsT=cat[jj][:, qsl],
                                rhs=wo_tiles[(jj, eb)][:, :],
                                start=(jj == 0),
                                stop=(jj == NDT - 1),
                            )
                        ost = p_out.tile([128, 512], f32, tag="outst", name=f"o{qt}_{eb}")
                        nc.vector.tensor_copy(out=ost[:, :], in_=ps[:, :])
                        nc.sync.dma_start(
                            out=out[qsl, 512 * eb : 512 * (eb + 1)], in_=ost[:, :]
                        )

            # --- phase 2: attention per head pair, software-pipelined ---
            # Within a pair, scores+exp for q-block `it` run one iteration
            # ahead of the attn@V (aug) matmuls for `it-1`, and normalization
            # for `it-2` trails another iteration.  The PE therefore always
            # has ready work queued and never idles long enough for the HAM
            # clock gate to re-throttle it to 1.2 GHz.
            for j in range(NDT):
                hA, hB = 2 * j, 2 * j + 1
                qtj, ktj = QT[j], KT[j]
                interleave = pending.pop(j + 1, [])
                if j == NDT - 1:
                    load_wo()
                cat[j] = p_cat.tile([128, S], f32r, tag="cat", name=f"cat{j}")
                pts = {}
                augs = {}
                s2s = {}

                def scores_exp(qb, g):
                    qsl = slice(512 * qb, 512 * (qb + 1))
                    psA = ps_mm.tile([128, 1024], f32, tag="mm", name=f"psA{j}{qb}{g}")
                    psB = ps_mm.tile([128, 1024], f32, tag="mm", name=f"psB{j}{qb}{g}")
                    for i in range(2):
                        kt = 2 * g + i
                        ksl = slice(128 * kt, 128 * (kt + 1))
                        osl = slice(512 * i, 512 * (i + 1))
                        nc.tensor.matmul(
                            psA[:, osl], lhsT=ktj[0:64, ksl], rhs=qtj[0:64, qsl],
                            start=True, stop=True,
                        )
                        nc.tensor.matmul(
                            psB[:, osl], lhsT=ktj[64:128, ksl], rhs=qtj[64:128, qsl],
                            start=True, stop=True,
                        )
                    ptA = p_pt.tile([128, 1024], bf16, tag="pt", name=f"ptA{j}{qb}{g}")
                    ptB = p_pt.tile([128, 1024], bf16, tag="pt", name=f"ptB{j}{qb}{g}")
                    nc.scalar.activation(out=ptA[:, :], in_=psA[:, :], func=EXP, scale=0.125)
                    nc.scalar.activation(out=ptB[:, :], in_=psB[:, :], func=EXP, scale=0.125)
                    pts[(qb, g)] = (ptA, ptB)

                def aug_mms(qb, g):
                    if qb not in augs:
                        augs[qb] = (
                            ps_aug.tile([65, 512], f32, tag="aug", name=f"augA{j}_{qb}"),
                            ps_aug.tile([65, 512], f32, tag="aug", name=f"augB{j}_{qb}"),
                        )
                    augA, augB = augs[qb]
                    ptA, ptB = pts.pop((qb, g))
                    for i in range(2):
                        kt = 2 * g + i
                        nc.tensor.matmul(
                            augA[:, :], lhsT=V[kt][:, 65 * hA : 65 * hA + 65],
                            rhs=ptA[:, 512 * i : 512 * (i + 1)],
                            start=(kt == 0), stop=(kt == NKT - 1),
                        )
                        nc.tensor.matmul(
                            augB[:, :], lhsT=V[kt][:, 65 * hB : 65 * hB + 65],
                            rhs=ptB[:, 512 * i : 512 * (i + 1)],
                            start=(kt == 0), stop=(kt == NKT - 1),
                        )

                def norm_gather(qb):
                    # pull the two softmax-denominator rows into SBUF (DVE),
                    # then assemble them on adjacent partitions via tiny DMAs
                    # (DVE cannot write to partition 1, DMA can)
                    augA, augB = augs[qb]
                    sA = p_recip.tile([1, 512], f32r, tag="sA", name=f"sA{j}_{qb}")
                    sB = p_recip.tile([1, 512], f32r, tag="sB", name=f"sB{j}_{qb}")
                    nc.vector.tensor_copy(out=sA[:, :], in_=augA[64:65, :])
                    nc.vector.tensor_copy(out=sB[:, :], in_=augB[64:65, :])
                    s2 = p_recip.tile([2, 512], f32r, tag="s2", name=f"s2_{j}_{qb}")
                    nc.sync.dma_start(out=s2[0:1, :], in_=sA[:, :])
                    nc.sync.dma_start(out=s2[1:2, :], in_=sB[:, :])
                    s2s[qb] = s2

                def norm_apply(qb):
                    # broadcast sums along partitions (PE outer product),
                    # reciprocal on DVE, then scale into catT
                    qsl = slice(512 * qb, 512 * (qb + 1))
                    s2 = s2s.pop(qb)
                    bc = ps_mm.tile([128, 512], f32, tag="mm", name=f"bc{j}_{qb}")
                    nc.tensor.matmul(
                        bc[:, :], lhsT=ind2[:, :], rhs=s2[:, :], start=True, stop=True
                    )
                    rec = p_recip.tile([128, 512], f32, tag="rec", name=f"rec{j}_{qb}")
                    nc.vector.reciprocal(out=rec[:, :], in_=bc[:, :])
                    augA, augB = augs.pop(qb)
                    nc.vector.tensor_mul(
                        out=cat[j][0:64, qsl], in0=augA[0:64, :], in1=rec[0:64, :]
                    )
                    nc.vector.tensor_mul(
                        out=cat[j][64:128, qsl], in0=augB[0:64, :], in1=rec[64:128, :]
                    )
                    if j == NDT - 1:
                        wo_chains(qb)

                for it in range(NQB + 2):
                    if it <= NQB:
                        for g in range(8):
                            if it < NQB:
                                scores_exp(it, g)
                            if 1 <= it:
                                aug_mms(it - 1, g)
                            if 2 <= it and g == 4 and (it - 2) in s2s:
                                norm_apply(it - 2)
                            if g % 2 == 1 and interleave:
                                interleave.pop(0)()
                        if 1 <= it:
                            norm_gather(it - 1)
                    else:
                        norm_apply(it - 2)
                # anything not interleaved (shouldn't happen, but be safe)
                for t in interleave:
                    t()

    _split_matmul_waits(nc)
    return nc


_SPLIT_TYPES = {"InstMatmult", "InstDMACopy", "InstActivation", "InstTensorCopy", "InstTensorTensor", "InstMemSet", "InstTensorScalarPtr", "InstTensorReduce", "InstReciprocal", "InstDrain", "InstNoOp", "InstEventSemaphore"}


def _split_matmul_waits(nc):
    """Several walrus instruction structs (fused-weight-load matmul S3_LW,
    DMA_DIRECT2D, ...) accept only one sync wait.  Move extra waits onto
    standalone no-ops on the same engine placed just before the instruction."""
    import concourse.mybir as mybir

    noop_cls = None
    for f in nc.m.functions:
        for blk in f.blocks:
            patched = []
            for inst in blk.instructions:
                si = getattr(inst, "sync_info", None)
                if (
                    type(inst).__name__ in _SPLIT_TYPES
                    and si is not None
                    and si.on_wait
                    and len(si.on_wait) > 1
                ):
                    if noop_cls is None:
                        import bass_rust

                        noop_cls = bass_rust.InstNoOp
                    waits = list(si.on_wait)
                    for w in waits[:-1]:
                        nop = noop_cls(
                            name=f"I-wsplit-{nc.next_id()}",
                            engine=inst.engine,
                            ins=[],
                            outs=[],
                        )
                        nop.sync_info = mybir.SyncInfo(on_wait=[w], on_update=[])
                        patched.append(nop)
                    inst.sync_info = mybir.SyncInfo(
                        on_wait=[waits[-1]], on_update=si.on_update
                    )
                patched.append(inst)
            blk.instructions[:] = patched


_CACHE = {}

import ml_dtypes

_BF16 = ml_dtypes.bfloat16

_IND2 = np.zeros((2, 128), dtype=np.float32)
_IND2[0, 0:64] = 1.0
_IND2[1, 64:128] = 1.0


def kernel(x, W_q, W_k, W_v, W_o):
    x = np.asarray(x, dtype=np.float32)
    W_q = np.asarray(W_q, dtype=np.float32)
    W_k = np.asarray(W_k, dtype=np.float32)
    W_v = np.asarray(W_v, dtype=np.float32)
    W_o = np.asarray(W_o, dtype=np.float32)

    if "nc" not in _CACHE:
        _CACHE["nc"] = build_bass()
    nc = _CACHE["nc"]

    from concourse.bass_utils import run_bass_kernel_spmd

    in_maps = []
    for c in range(8):
        b, j = divmod(c, 2)
        sl = slice(512 * j, 512 * (j + 1))
        in_maps.append(
            {
                "xT": np.ascontiguousarray(x[b].T),
                "wqT": np.ascontiguousarray(W_q[sl, :].T),
                "wkT": np.ascontiguousarray(W_k[sl, :].T),
                "wvT": np.ascontiguousarray(W_v[sl, :].T),
                "woT": np.ascontiguousarray(W_o[:, sl].T),
                "ones128": np.ones((128, 64), dtype=_BF16),
                "ind2": _IND2,
            }
        )

    res = run_bass_kernel_spmd(nc, in_maps, list(range(8))).results
    out = np.empty((4, S, C), dtype=np.float32)
    for b in range(4):
        out[b] = res[2 * b]["out"] + res[2 * b + 1]["out"]
    return out


# revision 21
# speedup vs baseline: 1.3836x; 1.0535x over previous
"""Multi-head attention (B=4, S=2048, D=1024, H=16) on 8 TRN2 NeuronCores.

Sharding: core c = (batch b = c//2, head-half j = c%2).  Each core computes
attention for 8 heads of one batch plus its partial output projection
(row-parallel W_o); the host sums the two partials per batch (the
"all-reduce") and stacks batches.

Per-core device algorithm (all matmuls in float32r = full-rate TF32-like):
  QT[d,s] = (W_q_shard @ x^T)    via lhsT=wqT chunks, rhs=xT chunks
  KT[d,s] likewise; V[s,dv] via lhsT=xT chunks, rhs=wvT
  per head pair / q-block / k-tile:
    S_T[k,q] = KT_h^T-slice @ QT_h  (row-packed: 2 heads of K=64 share the PE)
    P_T = exp(S_T/8)               (ACT, PSUM->SBUF)
    aug[dv+1,q] += [V|1]^T @ P_T   (row 64 = softmax denominators)
  normalize: recip (DVE) -> PE rank-2 outer-product broadcast -> DVE mul
  out[q,e] partial = cat^T-slices @ woT  -> DMA out
"""

import sys

sys.path.insert(0, "/opt/trn_rl_repo")

import numpy as np

S = 2048          # sequence length
C = 1024          # model dim (contraction for projections)
DV = 512          # per-core head dims (8 heads x 64)
HL = 8            # local heads
DK = 64
NSB = 4           # s-blocks of 512
NCT = 8           # c-tiles of 128
NDT = 4           # d-tiles of 128 (= head pairs)
NKT = 16          # k-tiles of 128
NQB = 4           # q-blocks of 512
NQT = 16          # q-tiles of 128
NEB = 2           # e-blocks of 512 (output model dim 1024)


def build_bass(debug=False):
    import concourse.bass as bass
    import concourse.mybir as mybir
    from concourse.tile import TileContext

    f32 = mybir.dt.float32
    f32r = mybir.dt.float32r
    bf16 = mybir.dt.bfloat16
    EXP = mybir.ActivationFunctionType.Exp

    nc = bass.Bass(target_bir_lowering=False, debug=debug)

    xT = nc.declare_dram_parameter("xT", [C, S], bf16, isOutput=False)
    wqT = nc.declare_dram_parameter("wqT", [C, DV], bf16, isOutput=False)
    wkT = nc.declare_dram_parameter("wkT", [C, DV], bf16, isOutput=False)
    wvT = nc.declare_dram_parameter("wvT", [C, DV], bf16, isOutput=False)
    woT = nc.declare_dram_parameter("woT", [DV, C], bf16, isOutput=False)
    onesd = nc.declare_dram_parameter("ones128", [128, 64], mybir.dt.bfloat16, isOutput=False)
    indd = nc.declare_dram_parameter("ind2", [2, 128], f32, isOutput=False)
    out = nc.declare_dram_parameter("out", [S, C], f32, isOutput=True)

    with nc.allow_low_precision(reason="f32r (tf32-like) matmul inputs; tol 2e-2"), TileContext(nc) as tc:
        with (
            tc.tile_pool(name="xt", bufs=10) as p_xt,
            tc.tile_pool(name="w", bufs=25) as p_w,
            tc.tile_pool(name="qk", bufs=5) as p_qk,
            tc.tile_pool(name="v", bufs=16) as p_v,
            tc.tile_pool(name="pt", bufs=24) as p_pt,
            tc.tile_pool(name="cat", bufs=4) as p_cat,
            tc.tile_pool(name="outst", bufs=2) as p_out,
            tc.tile_pool(name="misc", bufs=1) as p_misc,
            tc.tile_pool(name="recip", bufs=2) as p_recip,
            tc.tile_pool(name="psmm", bufs=2, space="PSUM") as ps_mm,
            tc.tile_pool(name="psaug", bufs=4, space="PSUM") as ps_aug,
        ):
            # --- constants ---
            ind2 = p_misc.tile([2, 128], f32r, tag="ind2", name="ind2")
            nc.sync.dma_start(out=ind2[:, :], in_=indd[:, :].bitcast(f32r))

            # --- weights (resident) ---
            wq = []
            wk = []
            wv = []
            for ct in range(NCT):
                tq = p_w.tile([128, DV], bf16, tag="w", name=f"wq{ct}")
                nc.sync.dma_start(out=tq[:, :], in_=wqT[128 * ct : 128 * (ct + 1), :])
                tk = p_w.tile([128, DV], bf16, tag="w", name=f"wk{ct}")
                nc.sync.dma_start(out=tk[:, :], in_=wkT[128 * ct : 128 * (ct + 1), :])
                tv = p_w.tile([128, DV], bf16, tag="w", name=f"wv{ct}")
                nc.sync.dma_start(out=tv[:, :], in_=wvT[128 * ct : 128 * (ct + 1), :])
                wq.append(tq)
                wk.append(tk)
                wv.append(tv)

            QT = [None] * NDT
            KT = [None] * NDT
            V = [None] * NKT
            cat = [None] * NDT

            def load_xchunks(sb):
                xc = []
                for ct in range(NCT):
                    t = p_xt.tile([128, 512], bf16, tag="xt", name=f"x{sb}_{ct}")
                    nc.sync.dma_start(
                        out=t[:, :],
                        in_=xT[128 * ct : 128 * (ct + 1), 512 * sb : 512 * (sb + 1)],
                    )
                    xc.append(t)
                return xc

            def qk_chain(dt, sb, xc):
                """Q and K projection chains for d-tile dt over s-block sb."""
                if QT[dt] is None:
                    QT[dt] = p_qk.tile([128, S], bf16, tag="qk", name=f"qt{dt}")
                    KT[dt] = p_qk.tile([128, S], bf16, tag="qk", name=f"kt{dt}")
                ps = ps_mm.tile([128, 1024], f32, tag="mm", name=f"psqk{dt}_{sb}")
                dsl = slice(128 * dt, 128 * (dt + 1))
                for ct in range(NCT):
                    nc.tensor.matmul(
                        ps[:, 0:512],
                        lhsT=wq[ct][:, dsl],
                        rhs=xc[ct][:, :],
                        start=(ct == 0),
                        stop=(ct == NCT - 1),
                    )
                for ct in range(NCT):
                    nc.tensor.matmul(
                        ps[:, 512:1024],
                        lhsT=wk[ct][:, dsl],
                        rhs=xc[ct][:, :],
                        start=(ct == 0),
                        stop=(ct == NCT - 1),
                    )
                ssl = slice(512 * sb, 512 * (sb + 1))
                nc.vector.tensor_copy(out=QT[dt][:, ssl], in_=ps[:, 0:512])
                nc.vector.tensor_copy(out=KT[dt][:, ssl], in_=ps[:, 512:1024])

            def v_chains(sb, xc):
                """V projection for the 4 s-tiles of s-block sb."""
                for half in range(2):
                    ps = ps_mm.tile([128, 1024], f32, tag="mm", name=f"psv{sb}_{half}")
                    for loc in range(2):
                        stl = 2 * half + loc
                        for ct in range(NCT):
                            nc.tensor.matmul(
                                ps[:, 512 * loc : 512 * (loc + 1)],
                                lhsT=xc[ct][:, 128 * stl : 128 * (stl + 1)],
                                rhs=wv[ct][:, :],
                                start=(ct == 0),
                                stop=(ct == NCT - 1),
                            )
                    for loc in range(2):
                        st = 4 * sb + 2 * half + loc
                        vt = p_v.tile([128, HL * 65], bf16, tag="v", name=f"v{st}")
                        V[st] = vt
                        # ones in column 64 of each head's 65-wide strip
                        nc.sync.dma_start(
                            out=vt[:, :].rearrange("p (h x) -> p h x", x=65)[:, :, 64:65],
                            in_=onesd[:, 0:HL],
                        )
                        nc.vector.tensor_copy(
                            out=vt[:, :].rearrange("p (h x) -> p h x", x=65)[:, :, 0:64],
                            in_=ps[:, 512 * loc : 512 * (loc + 1)].rearrange(
                                "p (h x) -> p h x", x=64
                            ),
                        )

            # --- phase 1: V + Q/K for d-tile 0 ---
            for sb in range(NSB):
                xc = load_xchunks(sb)
                v_chains(sb, xc)
                qk_chain(0, sb, xc)

            # deferred QK-projection tasks for later head pairs, interleaved
            # into the previous pair's attention to keep ACT fed
            def make_qk_tasks(dt):
                def task(sb=None, dt=dt):
                    xc = load_xchunks(sb)
                    qk_chain(dt, sb, xc)

                return [lambda sb=sb: task(sb) for sb in range(NSB)]

            pending = {j: make_qk_tasks(j) for j in range(1, NDT)}

            # deferred W_o chains (by q-block), interleaved into pair-3
            wo_tiles = {}

            def load_wo():
                for jj in range(NDT):
                    for eb in range(NEB):
                        t = p_w.tile([128, 512], bf16, tag="w", name=f"wo{jj}_{eb}")
                        nc.sync.dma_start(
                            out=t[:, :],
                            in_=woT[
                                128 * jj : 128 * (jj + 1), 512 * eb : 512 * (eb + 1)
                            ],
                        )
                        wo_tiles[(jj, eb)] = t

            wo_queue = []

            def wo_one(qt, eb):
                """One output-projection chain: out tile [128q, 512e]."""
                qsl = slice(128 * qt, 128 * (qt + 1))
                ps = ps_mm.tile([128, 512], f32, tag="mm", name=f"pso{qt}_{eb}")
                for jj in range(NDT):
                    nc.tensor.matmul(
                        ps[:, :],
                        lhsT=cat[jj][:, qsl],
                        rhs=wo_tiles[(jj, eb)][:, :],
                        start=(jj == 0),
                        stop=(jj == NDT - 1),
                    )
                ost = p_out.tile([128, 512], f32, tag="outst", name=f"o{qt}_{eb}")
                nc.vector.tensor_copy(out=ost[:, :], in_=ps[:, :])
                nc.sync.dma_start(
                    out=out[qsl, 512 * eb : 512 * (eb + 1)], in_=ost[:, :]
                )

            def wo_chains(qb):
                for qtl in range(4):
                    for eb in range(NEB):
                        wo_queue.append(
                            lambda qt=4 * qb + qtl, eb=eb: wo_one(qt, eb)
                        )

            # --- phase 2: attention per head pair, software-pipelined ---
            # Within a pair, scores+exp for q-block `it` run one iteration
            # ahead of the attn@V (aug) matmuls for `it-1`, and normalization
            # for `it-2` trails another iteration.  The PE therefore always
            # has ready work queued and never idles long enough for the HAM
            # clock gate to re-throttle it to 1.2 GHz.
            for j in range(NDT):
                hA, hB = 2 * j, 2 * j + 1
                qtj, ktj = QT[j], KT[j]
                interleave = pending.pop(j + 1, [])
                if j == NDT - 1:
                    load_wo()
                cat[j] = p_cat.tile([128, S], bf16, tag="cat", name=f"cat{j}")
                pts = {}
                augs = {}
                s2s = {}

                def scores_exp(qb, g):
                    qsl = slice(512 * qb, 512 * (qb + 1))
                    psA = ps_mm.tile([128, 1024], f32, tag="mm", name=f"psA{j}{qb}{g}")
                    psB = ps_mm.tile([128, 1024], f32, tag="mm", name=f"psB{j}{qb}{g}")
                    for i in range(2):
                        kt = 2 * g + i
                        ksl = slice(128 * kt, 128 * (kt + 1))
                        osl = slice(512 * i, 512 * (i + 1))
                        nc.tensor.matmul(
                            psA[:, osl], lhsT=ktj[0:64, ksl], rhs=qtj[0:64, qsl],
                            start=True, stop=True,
                        )
                        nc.tensor.matmul(
                            psB[:, osl], lhsT=ktj[64:128, ksl], rhs=qtj[64:128, qsl],
                            start=True, stop=True,
                        )
                    ptA = p_pt.tile([128, 1024], bf16, tag="pt", name=f"ptA{j}{qb}{g}")
                    ptB = p_pt.tile([128, 1024], bf16, tag="pt", name=f"ptB{j}{qb}{g}")
                    nc.scalar.activation(out=ptA[:, :], in_=psA[:, :], func=EXP, scale=0.125)
                    nc.scalar.activation(out=ptB[:, :], in_=psB[:, :], func=EXP, scale=0.125)
                    pts[(qb, g)] = (ptA, ptB)

                def aug_mms(qb, g):
                    if qb not in augs:
                        augs[qb] = (
                            ps_aug.tile([65, 512], f32, tag="aug", name=f"augA{j}_{qb}"),
                            ps_aug.tile([65, 512], f32, tag="aug", name=f"augB{j}_{qb}"),
                        )
                    augA, augB = augs[qb]
                    ptA, ptB = pts.pop((qb, g))
                    for i in range(2):
                        kt = 2 * g + i
                        nc.tensor.matmul(
                            augA[:, :], lhsT=V[kt][:, 65 * hA : 65 * hA + 65],
                            rhs=ptA[:, 512 * i : 512 * (i + 1)],
                            start=(kt == 0), stop=(kt == NKT - 1),
                        )
                        nc.tensor.matmul(
                            augB[:, :], lhsT=V[kt][:, 65 * hB : 65 * hB + 65],
                            rhs=ptB[:, 512 * i : 512 * (i + 1)],
                            start=(kt == 0), stop=(kt == NKT - 1),
                        )

                def norm_gather(qb):
                    # pull the two softmax-denominator rows into SBUF (DVE),
                    # then assemble them on adjacent partitions via tiny DMAs
                    # (DVE cannot write to partition 1, DMA can)
                    augA, augB = augs[qb]
                    sA = p_recip.tile([1, 512], f32r, tag="sA", name=f"sA{j}_{qb}")
                    sB = p_recip.tile([1, 512], f32r, tag="sB", name=f"sB{j}_{qb}")
                    nc.vector.tensor_copy(out=sA[:, :], in_=augA[64:65, :])
                    nc.vector.tensor_copy(out=sB[:, :], in_=augB[64:65, :])
                    s2 = p_recip.tile([2, 512], f32r, tag="s2", name=f"s2_{j}_{qb}")
                    nc.sync.dma_start(out=s2[0:1, :], in_=sA[:, :])
                    nc.sync.dma_start(out=s2[1:2, :], in_=sB[:, :])
                    s2s[qb] = s2

                def norm_apply(qb):
                    # broadcast sums along partitions (PE outer product),
                    # reciprocal on DVE, then scale into catT
                    qsl = slice(512 * qb, 512 * (qb + 1))
                    s2 = s2s.pop(qb)
                    bc = ps_mm.tile([128, 512], f32, tag="mm", name=f"bc{j}_{qb}")
                    nc.tensor.matmul(
                        bc[:, :], lhsT=ind2[:, :], rhs=s2[:, :], start=True, stop=True
                    )
                    rec = p_recip.tile([128, 512], f32, tag="rec", name=f"rec{j}_{qb}")
                    nc.vector.reciprocal(out=rec[:, :], in_=bc[:, :])
                    augA, augB = augs.pop(qb)
                    nc.vector.tensor_mul(
                        out=cat[j][0:64, qsl], in0=augA[0:64, :], in1=rec[0:64, :]
                    )
                    nc.vector.tensor_mul(
                        out=cat[j][64:128, qsl], in0=augB[0:64, :], in1=rec[64:128, :]
                    )
                    if j == NDT - 1:
                        wo_chains(qb)

                for it in range(NQB + 2):
                    if it <= NQB:
                        for g in range(8):
                            if it < NQB:
                                scores_exp(it, g)
                            if 1 <= it:
                                aug_mms(it - 1, g)
                            if 2 <= it and g == 4 and (it - 2) in s2s:
                                norm_apply(it - 2)
                            if g % 2 == 1 and interleave:
                                interleave.pop(0)()
                            if wo_queue:
                                wo_queue.pop(0)()
                        if 1 <= it:
                            norm_gather(it - 1)
                    else:
                        norm_apply(it - 2)
                # anything not interleaved (shouldn't happen, but be safe)
                for t in interleave:
                    t()
            for t in wo_queue:
                t()

    _split_matmul_waits(nc)
    return nc


_SPLIT_TYPES = {"InstMatmult", "InstDMACopy", "InstActivation", "InstTensorCopy", "InstTensorTensor", "InstMemSet", "InstTensorScalarPtr", "InstTensorReduce", "InstReciprocal", "InstDrain", "InstNoOp", "InstEventSemaphore"}


def _split_matmul_waits(nc):
    """Several walrus instruction structs (fused-weight-load matmul S3_LW,
    DMA_DIRECT2D, ...) accept only one sync wait.  Move extra waits onto
    standalone no-ops on the same engine placed just before the instruction."""
    import concourse.mybir as mybir

    noop_cls = None
    for f in nc.m.functions:
        for blk in f.blocks:
            patched = []
            for inst in blk.instructions:
                si = getattr(inst, "sync_info", None)
                if (
                    type(inst).__name__ in _SPLIT_TYPES
                    and si is not None
                    and si.on_wait
                    and len(si.on_wait) > 1
                ):
                    if noop_cls is None:
                        import bass_rust

                        noop_cls = bass_rust.InstNoOp
                    waits = list(si.on_wait)
                    for w in waits[:-1]:
                        nop = noop_cls(
                            name=f"I-wsplit-{nc.next_id()}",
                            engine=inst.engine,
                            ins=[],
                            outs=[],
                        )
                        nop.sync_info = mybir.SyncInfo(on_wait=[w], on_update=[])
                        patched.append(nop)
                    inst.sync_info = mybir.SyncInfo(
                        on_wait=[waits[-1]], on_update=si.on_update
                    )
                patched.append(inst)
            blk.instructions[:] = patched


_CACHE = {}

import ml_dtypes

_BF16 = ml_dtypes.bfloat16

_IND2 = np.zeros((2, 128), dtype=np.float32)
_IND2[0, 0:64] = 1.0
_IND2[1, 64:128] = 1.0


def kernel(x, W_q, W_k, W_v, W_o):
    x = np.asarray(x, dtype=np.float32)
    W_q = np.asarray(W_q, dtype=np.float32)
    W_k = np.asarray(W_k, dtype=np.float32)
    W_v = np.asarray(W_v, dtype=np.float32)
    W_o = np.asarray(W_o, dtype=np.float32)

    if "nc" not in _CACHE:
        _CACHE["nc"] = build_bass()
    nc = _CACHE["nc"]

    from concourse.bass_utils import run_bass_kernel_spmd

    in_maps = []
    for c in range(8):
        b, j = divmod(c, 2)
        sl = slice(512 * j, 512 * (j + 1))
        in_maps.append(
            {
                "xT": np.ascontiguousarray(x[b].T).astype(_BF16),
                "wqT": np.ascontiguousarray(W_q[sl, :].T).astype(_BF16),
                "wkT": np.ascontiguousarray(W_k[sl, :].T).astype(_BF16),
                "wvT": np.ascontiguousarray(W_v[sl, :].T).astype(_BF16),
                "woT": np.ascontiguousarray(W_o[:, sl].T).astype(_BF16),
                "ones128": np.ones((128, 64), dtype=_BF16),
                "ind2": _IND2,
            }
        )

    res = run_bass_kernel_spmd(nc, in_maps, list(range(8))).results
    out = np.empty((4, S, C), dtype=np.float32)
    for b in range(4):
        out[b] = res[2 * b]["out"] + res[2 * b + 1]["out"]
    return out


# revision 22
# speedup vs baseline: 1.4119x; 1.0205x over previous
"""Multi-head attention (B=4, S=2048, D=1024, H=16) on 8 TRN2 NeuronCores.

Sharding: core c = (batch b = c//2, head-half j = c%2).  Each core computes
attention for 8 heads of one batch plus its partial output projection
(row-parallel W_o); the host sums the two partials per batch (the
"all-reduce") and stacks batches.

Per-core device algorithm (all matmuls in float32r = full-rate TF32-like):
  QT[d,s] = (W_q_shard @ x^T)    via lhsT=wqT chunks, rhs=xT chunks
  KT[d,s] likewise; V[s,dv] via lhsT=xT chunks, rhs=wvT
  per head pair / q-block / k-tile:
    S_T[k,q] = KT_h^T-slice @ QT_h  (row-packed: 2 heads of K=64 share the PE)
    P_T = exp(S_T/8)               (ACT, PSUM->SBUF)
    aug[dv+1,q] += [V|1]^T @ P_T   (row 64 = softmax denominators)
  normalize: recip (DVE) -> PE rank-2 outer-product broadcast -> DVE mul
  out[q,e] partial = cat^T-slices @ woT  -> DMA out
"""

import sys

sys.path.insert(0, "/opt/trn_rl_repo")

import numpy as np

S = 2048          # sequence length
C = 1024          # model dim (contraction for projections)
DV = 512          # per-core head dims (8 heads x 64)
HL = 8            # local heads
DK = 64
NSB = 4           # s-blocks of 512
NCT = 8           # c-tiles of 128
NDT = 4           # d-tiles of 128 (= head pairs)
NKT = 16          # k-tiles of 128
NQB = 4           # q-blocks of 512
NQT = 16          # q-tiles of 128
NEB = 2           # e-blocks of 512 (output model dim 1024)


def build_bass(debug=False):
    import concourse.bass as bass
    import concourse.mybir as mybir
    from concourse.tile import TileContext

    f32 = mybir.dt.float32
    f32r = mybir.dt.float32r
    bf16 = mybir.dt.bfloat16
    EXP = mybir.ActivationFunctionType.Exp

    nc = bass.Bass(target_bir_lowering=False, debug=debug)

    xT = nc.declare_dram_parameter("xT", [C, S], bf16, isOutput=False)
    wqT = nc.declare_dram_parameter("wqT", [C, DV], bf16, isOutput=False)
    wkT = nc.declare_dram_parameter("wkT", [C, DV], bf16, isOutput=False)
    wvT = nc.declare_dram_parameter("wvT", [C, DV], bf16, isOutput=False)
    woT = nc.declare_dram_parameter("woT", [DV, C], bf16, isOutput=False)
    onesd = nc.declare_dram_parameter("ones128", [128, 64], mybir.dt.bfloat16, isOutput=False)
    indd = nc.declare_dram_parameter("ind2", [2, 128], f32, isOutput=False)
    out = nc.declare_dram_parameter("out", [S, C], f32, isOutput=True)

    with nc.allow_low_precision(reason="f32r (tf32-like) matmul inputs; tol 2e-2"), TileContext(nc) as tc:
        with (
            tc.tile_pool(name="xt", bufs=10) as p_xt,
            tc.tile_pool(name="w", bufs=25) as p_w,
            tc.tile_pool(name="qk", bufs=5) as p_qk,
            tc.tile_pool(name="v", bufs=16) as p_v,
            tc.tile_pool(name="pt", bufs=24) as p_pt,
            tc.tile_pool(name="cat", bufs=4) as p_cat,
            tc.tile_pool(name="outst", bufs=2) as p_out,
            tc.tile_pool(name="misc", bufs=1) as p_misc,
            tc.tile_pool(name="recip", bufs=2) as p_recip,
            tc.tile_pool(name="psmm", bufs=2, space="PSUM") as ps_mm,
            tc.tile_pool(name="psaug", bufs=4, space="PSUM") as ps_aug,
        ):
            # --- constants ---
            ind2 = p_misc.tile([2, 128], f32r, tag="ind2", name="ind2")
            nc.sync.dma_start(out=ind2[:, :], in_=indd[:, :].bitcast(f32r))

            # --- weights (resident) ---
            wq = []
            wk = []
            wv = []
            for ct in range(NCT):
                tq = p_w.tile([128, DV], bf16, tag="w", name=f"wq{ct}")
                nc.sync.dma_start(out=tq[:, :], in_=wqT[128 * ct : 128 * (ct + 1), :])
                tk = p_w.tile([128, DV], bf16, tag="w", name=f"wk{ct}")
                nc.sync.dma_start(out=tk[:, :], in_=wkT[128 * ct : 128 * (ct + 1), :])
                tv = p_w.tile([128, DV], bf16, tag="w", name=f"wv{ct}")
                nc.sync.dma_start(out=tv[:, :], in_=wvT[128 * ct : 128 * (ct + 1), :])
                wq.append(tq)
                wk.append(tk)
                wv.append(tv)

            QT = [None] * NDT
            KT = [None] * NDT
            V = [None] * NKT
            cat = [None] * NDT

            def load_xchunks(sb):
                xc = []
                for ct in range(NCT):
                    t = p_xt.tile([128, 512], bf16, tag="xt", name=f"x{sb}_{ct}")
                    nc.sync.dma_start(
                        out=t[:, :],
                        in_=xT[128 * ct : 128 * (ct + 1), 512 * sb : 512 * (sb + 1)],
                    )
                    xc.append(t)
                return xc

            def qk_chain(dt, sb, xc):
                """Q and K projection chains for d-tile dt over s-block sb."""
                if QT[dt] is None:
                    QT[dt] = p_qk.tile([128, S], bf16, tag="qk", name=f"qt{dt}")
                    KT[dt] = p_qk.tile([128, S], bf16, tag="qk", name=f"kt{dt}")
                ps = ps_mm.tile([128, 1024], f32, tag="mm", name=f"psqk{dt}_{sb}")
                dsl = slice(128 * dt, 128 * (dt + 1))
                for ct in range(NCT):
                    nc.tensor.matmul(
                        ps[:, 0:512],
                        lhsT=wq[ct][:, dsl],
                        rhs=xc[ct][:, :],
                        start=(ct == 0),
                        stop=(ct == NCT - 1),
                    )
                for ct in range(NCT):
                    nc.tensor.matmul(
                        ps[:, 512:1024],
                        lhsT=wk[ct][:, dsl],
                        rhs=xc[ct][:, :],
                        start=(ct == 0),
                        stop=(ct == NCT - 1),
                    )
                ssl = slice(512 * sb, 512 * (sb + 1))
                nc.vector.tensor_copy(out=QT[dt][:, ssl], in_=ps[:, 0:512])
                nc.vector.tensor_copy(out=KT[dt][:, ssl], in_=ps[:, 512:1024])

            def v_chains(sb, xc):
                """V projection for the 4 s-tiles of s-block sb."""
                for half in range(2):
                    ps = ps_mm.tile([128, 1024], f32, tag="mm", name=f"psv{sb}_{half}")
                    for loc in range(2):
                        stl = 2 * half + loc
                        for ct in range(NCT):
                            nc.tensor.matmul(
                                ps[:, 512 * loc : 512 * (loc + 1)],
                                lhsT=xc[ct][:, 128 * stl : 128 * (stl + 1)],
                                rhs=wv[ct][:, :],
                                start=(ct == 0),
                                stop=(ct == NCT - 1),
                            )
                    for loc in range(2):
                        st = 4 * sb + 2 * half + loc
                        vt = p_v.tile([128, HL * 65], bf16, tag="v", name=f"v{st}")
                        V[st] = vt
                        # ones in column 64 of each head's 65-wide strip
                        nc.sync.dma_start(
                            out=vt[:, :].rearrange("p (h x) -> p h x", x=65)[:, :, 64:65],
                            in_=onesd[:, 0:HL],
                        )
                        nc.vector.tensor_copy(
                            out=vt[:, :].rearrange("p (h x) -> p h x", x=65)[:, :, 0:64],
                            in_=ps[:, 512 * loc : 512 * (loc + 1)].rearrange(
                                "p (h x) -> p h x", x=64
                            ),
                        )

            # --- phase 1: V + Q/K for d-tile 0 ---
            for sb in range(NSB):
                xc = load_xchunks(sb)
                v_chains(sb, xc)
                qk_chain(0, sb, xc)

            # deferred QK-projection tasks for later head pairs, interleaved
            # into the previous pair's attention to keep ACT fed
            def make_qk_tasks(dt):
                def task(sb=None, dt=dt):
                    xc = load_xchunks(sb)
                    qk_chain(dt, sb, xc)

                return [lambda sb=sb: task(sb) for sb in range(NSB)]

            pending = {j: make_qk_tasks(j) for j in range(1, NDT)}

            # deferred W_o chains (by q-block), interleaved into pair-3
            wo_tiles = {}

            def load_wo():
                for jj in range(NDT):
                    for eb in range(NEB):
                        t = p_w.tile([128, 512], bf16, tag="w", name=f"wo{jj}_{eb}")
                        nc.sync.dma_start(
                            out=t[:, :],
                            in_=woT[
                                128 * jj : 128 * (jj + 1), 512 * eb : 512 * (eb + 1)
                            ],
                        )
                        wo_tiles[(jj, eb)] = t

            wo_queue = []

            def wo_one(qt, eb):
                """One output-projection chain: out tile [128q, 512e]."""
                qsl = slice(128 * qt, 128 * (qt + 1))
                ps = ps_mm.tile([128, 512], f32, tag="mm", name=f"pso{qt}_{eb}")
                for jj in range(NDT):
                    nc.tensor.matmul(
                        ps[:, :],
                        lhsT=cat[jj][:, qsl],
                        rhs=wo_tiles[(jj, eb)][:, :],
                        start=(jj == 0),
                        stop=(jj == NDT - 1),
                    )
                ost = p_out.tile([128, 512], f32, tag="outst", name=f"o{qt}_{eb}")
                nc.vector.tensor_copy(out=ost[:, :], in_=ps[:, :])
                nc.sync.dma_start(
                    out=out[qsl, 512 * eb : 512 * (eb + 1)], in_=ost[:, :]
                )

            def wo_chains(qb):
                for qtl in range(4):
                    for eb in range(NEB):
                        wo_queue.append(
                            lambda qt=4 * qb + qtl, eb=eb: wo_one(qt, eb)
                        )

            # --- phase 2: attention per head pair, software-pipelined ---
            # Within a pair, scores+exp for q-block `it` run one iteration
            # ahead of the attn@V (aug) matmuls for `it-1`, and normalization
            # for `it-2` trails another iteration.  The PE therefore always
            # has ready work queued and never idles long enough for the HAM
            # clock gate to re-throttle it to 1.2 GHz.
            for j in range(NDT):
                cat[j] = p_cat.tile([128, S], bf16, tag="cat", name=f"cat{j}")
            load_wo()
            pts = {}
            augs = {}
            s2s = {}
            interleave = []

            def scores_exp(j, qb, g):
                qtj, ktj = QT[j], KT[j]
                qsl = slice(512 * qb, 512 * (qb + 1))
                psA = ps_mm.tile([128, 1024], f32, tag="mm", name=f"psA{j}{qb}{g}")
                psB = ps_mm.tile([128, 1024], f32, tag="mm", name=f"psB{j}{qb}{g}")
                for i in range(2):
                    kt = 2 * g + i
                    ksl = slice(128 * kt, 128 * (kt + 1))
                    osl = slice(512 * i, 512 * (i + 1))
                    nc.tensor.matmul(
                        psA[:, osl], lhsT=ktj[0:64, ksl], rhs=qtj[0:64, qsl],
                        start=True, stop=True,
                    )
                    nc.tensor.matmul(
                        psB[:, osl], lhsT=ktj[64:128, ksl], rhs=qtj[64:128, qsl],
                        start=True, stop=True,
                    )
                ptA = p_pt.tile([128, 1024], bf16, tag="pt", name=f"ptA{j}{qb}{g}")
                ptB = p_pt.tile([128, 1024], bf16, tag="pt", name=f"ptB{j}{qb}{g}")
                nc.scalar.activation(out=ptA[:, :], in_=psA[:, :], func=EXP, scale=0.125)
                nc.scalar.activation(out=ptB[:, :], in_=psB[:, :], func=EXP, scale=0.125)
                pts[(j, qb, g)] = (ptA, ptB)

            def aug_mms(j, qb, g):
                hA, hB = 2 * j, 2 * j + 1
                if (j, qb) not in augs:
                    augs[(j, qb)] = (
                        ps_aug.tile([65, 512], f32, tag="aug", name=f"augA{j}_{qb}"),
                        ps_aug.tile([65, 512], f32, tag="aug", name=f"augB{j}_{qb}"),
                    )
                augA, augB = augs[(j, qb)]
                ptA, ptB = pts.pop((j, qb, g))
                for i in range(2):
                    kt = 2 * g + i
                    nc.tensor.matmul(
                        augA[:, :], lhsT=V[kt][:, 65 * hA : 65 * hA + 65],
                        rhs=ptA[:, 512 * i : 512 * (i + 1)],
                        start=(kt == 0), stop=(kt == NKT - 1),
                    )
                    nc.tensor.matmul(
                        augB[:, :], lhsT=V[kt][:, 65 * hB : 65 * hB + 65],
                        rhs=ptB[:, 512 * i : 512 * (i + 1)],
                        start=(kt == 0), stop=(kt == NKT - 1),
                    )

            def norm_gather(j, qb):
                # pull the two softmax-denominator rows into SBUF (DVE),
                # then assemble them on adjacent partitions via tiny DMAs
                # (DVE cannot write to partition 1, DMA can)
                augA, augB = augs[(j, qb)]
                sA = p_recip.tile([1, 512], f32r, tag="sA", name=f"sA{j}_{qb}")
                sB = p_recip.tile([1, 512], f32r, tag="sB", name=f"sB{j}_{qb}")
                nc.vector.tensor_copy(out=sA[:, :], in_=augA[64:65, :])
                nc.vector.tensor_copy(out=sB[:, :], in_=augB[64:65, :])
                s2 = p_recip.tile([2, 512], f32r, tag="s2", name=f"s2_{j}_{qb}")
                nc.sync.dma_start(out=s2[0:1, :], in_=sA[:, :])
                nc.sync.dma_start(out=s2[1:2, :], in_=sB[:, :])
                s2s[(j, qb)] = s2

            def norm_apply(j, qb):
                # broadcast sums along partitions (PE outer product),
                # reciprocal on DVE, then scale into catT
                qsl = slice(512 * qb, 512 * (qb + 1))
                s2 = s2s.pop((j, qb))
                bc = ps_mm.tile([128, 512], f32, tag="mm", name=f"bc{j}_{qb}")
                nc.tensor.matmul(
                    bc[:, :], lhsT=ind2[:, :], rhs=s2[:, :], start=True, stop=True
                )
                rec = p_recip.tile([128, 512], f32, tag="rec", name=f"rec{j}_{qb}")
                nc.vector.reciprocal(out=rec[:, :], in_=bc[:, :])
                augA, augB = augs.pop((j, qb))
                nc.vector.tensor_mul(
                    out=cat[j][0:64, qsl], in0=augA[0:64, :], in1=rec[0:64, :]
                )
                nc.vector.tensor_mul(
                    out=cat[j][64:128, qsl], in0=augB[0:64, :], in1=rec[64:128, :]
                )
                if j == NDT - 1:
                    wo_chains(qb)

            # one flat pipeline over all (pair, q-block) items: the aug
            # stream trails the scores stream by one item and crosses pair
            # boundaries without draining, so the PE stays dense end to end
            items = [(j, qb) for j in range(NDT) for qb in range(NQB)]
            NI = len(items)
            for idx in range(NI + 2):
                if idx < NI and items[idx][1] == 0 and items[idx][0] + 1 < NDT:
                    # queue projection chains for the next pair
                    interleave.extend(pending.pop(items[idx][0] + 1))
                for g in range(8):
                    if idx < NI:
                        scores_exp(*items[idx], g)
                    if 1 <= idx <= NI:
                        aug_mms(*items[idx - 1], g)
                    if 2 <= idx and g == 4 and items[idx - 2] in s2s:
                        norm_apply(*items[idx - 2])
                    if g % 2 == 1 and interleave:
                        interleave.pop(0)()
                    if wo_queue:
                        wo_queue.pop(0)()
                if 1 <= idx <= NI:
                    norm_gather(*items[idx - 1])
            for t in interleave:
                t()
            for t in wo_queue:
                t()

    _split_matmul_waits(nc)
    return nc


_SPLIT_TYPES = {"InstMatmult", "InstDMACopy", "InstActivation", "InstTensorCopy", "InstTensorTensor", "InstMemSet", "InstTensorScalarPtr", "InstTensorReduce", "InstReciprocal", "InstDrain", "InstNoOp", "InstEventSemaphore"}


def _split_matmul_waits(nc):
    """Several walrus instruction structs (fused-weight-load matmul S3_LW,
    DMA_DIRECT2D, ...) accept only one sync wait.  Move extra waits onto
    standalone no-ops on the same engine placed just before the instruction."""
    import concourse.mybir as mybir

    noop_cls = None
    for f in nc.m.functions:
        for blk in f.blocks:
            patched = []
            for inst in blk.instructions:
                si = getattr(inst, "sync_info", None)
                if (
                    type(inst).__name__ in _SPLIT_TYPES
                    and si is not None
                    and si.on_wait
                    and len(si.on_wait) > 1
                ):
                    if noop_cls is None:
                        import bass_rust

                        noop_cls = bass_rust.InstNoOp
                    waits = list(si.on_wait)
                    for w in waits[:-1]:
                        nop = noop_cls(
                            name=f"I-wsplit-{nc.next_id()}",
                            engine=inst.engine,
                            ins=[],
                            outs=[],
                        )
                        nop.sync_info = mybir.SyncInfo(on_wait=[w], on_update=[])
                        patched.append(nop)
                    inst.sync_info = mybir.SyncInfo(
                        on_wait=[waits[-1]], on_update=si.on_update
                    )
                patched.append(inst)
            blk.instructions[:] = patched


_CACHE = {}

import ml_dtypes

_BF16 = ml_dtypes.bfloat16

_IND2 = np.zeros((2, 128), dtype=np.float32)
_IND2[0, 0:64] = 1.0
_IND2[1, 64:128] = 1.0


def kernel(x, W_q, W_k, W_v, W_o):
    x = np.asarray(x, dtype=np.float32)
    W_q = np.asarray(W_q, dtype=np.float32)
    W_k = np.asarray(W_k, dtype=np.float32)
    W_v = np.asarray(W_v, dtype=np.float32)
    W_o = np.asarray(W_o, dtype=np.float32)

    if "nc" not in _CACHE:
        _CACHE["nc"] = build_bass()
    nc = _CACHE["nc"]

    from concourse.bass_utils import run_bass_kernel_spmd

    in_maps = []
    for c in range(8):
        b, j = divmod(c, 2)
        sl = slice(512 * j, 512 * (j + 1))
        in_maps.append(
            {
                "xT": np.ascontiguousarray(x[b].T).astype(_BF16),
                "wqT": np.ascontiguousarray(W_q[sl, :].T).astype(_BF16),
                "wkT": np.ascontiguousarray(W_k[sl, :].T).astype(_BF16),
                "wvT": np.ascontiguousarray(W_v[sl, :].T).astype(_BF16),
                "woT": np.ascontiguousarray(W_o[:, sl].T).astype(_BF16),
                "ones128": np.ones((128, 64), dtype=_BF16),
                "ind2": _IND2,
            }
        )

    res = run_bass_kernel_spmd(nc, in_maps, list(range(8))).results
    out = np.empty((4, S, C), dtype=np.float32)
    for b in range(4):
        out[b] = res[2 * b]["out"] + res[2 * b + 1]["out"]
    return out


# revision 25
# speedup vs baseline: 1.4628x; 1.0361x over previous
"""Multi-head attention (B=4, S=2048, D=1024, H=16) on 8 TRN2 NeuronCores.

Sharding: core c = (batch b = c//2, head-half j = c%2).  Each core computes
attention for 8 heads of one batch plus its partial output projection
(row-parallel W_o); the host sums the two partials per batch (the
"all-reduce") and stacks batches.

Per-core device algorithm (all matmuls in float32r = full-rate TF32-like):
  QT[d,s] = (W_q_shard @ x^T)    via lhsT=wqT chunks, rhs=xT chunks
  KT[d,s] likewise; V[s,dv] via lhsT=xT chunks, rhs=wvT
  per head pair / q-block / k-tile:
    S_T[k,q] = KT_h^T-slice @ QT_h  (row-packed: 2 heads of K=64 share the PE)
    P_T = exp(S_T/8)               (ACT, PSUM->SBUF)
    aug[dv+1,q] += [V|1]^T @ P_T   (row 64 = softmax denominators)
  normalize: recip (DVE) -> PE rank-2 outer-product broadcast -> DVE mul
  out[q,e] partial = cat^T-slices @ woT  -> DMA out
"""

import sys

sys.path.insert(0, "/opt/trn_rl_repo")

import numpy as np

S = 2048          # sequence length
C = 1024          # model dim (contraction for projections)
DV = 512          # per-core head dims (8 heads x 64)
HL = 8            # local heads
DK = 64
NSB = 4           # s-blocks of 512
NCT = 8           # c-tiles of 128
NDT = 4           # d-tiles of 128 (= head pairs)
NKT = 16          # k-tiles of 128
NQB = 4           # q-blocks of 512
NQT = 16          # q-tiles of 128
NEB = 2           # e-blocks of 512 (output model dim 1024)


def build_bass(debug=False):
    import concourse.bass as bass
    import concourse.mybir as mybir
    from concourse.tile import TileContext

    f32 = mybir.dt.float32
    f32r = mybir.dt.float32r
    bf16 = mybir.dt.bfloat16
    EXP = mybir.ActivationFunctionType.Exp

    nc = bass.Bass(target_bir_lowering=False, debug=debug)

    xT = nc.declare_dram_parameter("xT", [C, S], bf16, isOutput=False)
    wqT = nc.declare_dram_parameter("wqT", [C, DV], bf16, isOutput=False)
    wkT = nc.declare_dram_parameter("wkT", [C, DV], bf16, isOutput=False)
    wvT = nc.declare_dram_parameter("wvT", [C, DV], bf16, isOutput=False)
    woT = nc.declare_dram_parameter("woT", [DV, C], bf16, isOutput=False)
    onesd = nc.declare_dram_parameter("ones128", [128, 64], mybir.dt.bfloat16, isOutput=False)
    indd = nc.declare_dram_parameter("ind2", [2, 128], f32, isOutput=False)
    out = nc.declare_dram_parameter("out", [S, C], f32, isOutput=True)

    with nc.allow_low_precision(reason="f32r (tf32-like) matmul inputs; tol 2e-2"), TileContext(nc) as tc:
        with (
            tc.tile_pool(name="xt", bufs=10) as p_xt,
            tc.tile_pool(name="w", bufs=25) as p_w,
            tc.tile_pool(name="qk", bufs=5) as p_qk,
            tc.tile_pool(name="v", bufs=16) as p_v,
            tc.tile_pool(name="pt", bufs=24) as p_pt,
            tc.tile_pool(name="cat", bufs=4) as p_cat,
            tc.tile_pool(name="outst", bufs=2) as p_out,
            tc.tile_pool(name="misc", bufs=1) as p_misc,
            tc.tile_pool(name="recip", bufs=2) as p_recip,
            tc.tile_pool(name="acc", bufs=4) as p_acc,
            tc.tile_pool(name="psmm", bufs=3, space="PSUM") as ps_mm,
            tc.tile_pool(name="psaug", bufs=2, space="PSUM") as ps_aug,
        ):
            # --- constants ---
            ind2 = p_misc.tile([2, 128], f32r, tag="ind2", name="ind2")
            nc.sync.dma_start(out=ind2[:, :], in_=indd[:, :].bitcast(f32r))

            # --- weights: V first (critical path), Q/K just-in-time ---
            wq = []
            wk = []
            wv = []
            for ct in range(NCT):
                tv = p_w.tile([128, DV], bf16, tag="w", name=f"wv{ct}")
                nc.sync.dma_start(out=tv[:, :], in_=wvT[128 * ct : 128 * (ct + 1), :])
                wv.append(tv)

            def load_wqk():
                for ct in range(NCT):
                    tq = p_w.tile([128, DV], bf16, tag="w", name=f"wq{ct}")
                    nc.sync.dma_start(out=tq[:, :], in_=wqT[128 * ct : 128 * (ct + 1), :])
                    tk = p_w.tile([128, DV], bf16, tag="w", name=f"wk{ct}")
                    nc.sync.dma_start(out=tk[:, :], in_=wkT[128 * ct : 128 * (ct + 1), :])
                    wq.append(tq)
                    wk.append(tk)

            QT = [None] * NDT
            KT = [None] * NDT
            V = [None] * NKT
            cat = [None] * NDT

            def load_xchunks(sb):
                xc = []
                for ct in range(NCT):
                    t = p_xt.tile([128, 512], bf16, tag="xt", name=f"x{sb}_{ct}")
                    nc.sync.dma_start(
                        out=t[:, :],
                        in_=xT[128 * ct : 128 * (ct + 1), 512 * sb : 512 * (sb + 1)],
                    )
                    xc.append(t)
                return xc

            def qk_chain(dt, sb, xc):
                """Q and K projection chains for d-tile dt over s-block sb."""
                if QT[dt] is None:
                    QT[dt] = p_qk.tile([128, S], bf16, tag="qk", name=f"qt{dt}")
                    KT[dt] = p_qk.tile([128, S], bf16, tag="qk", name=f"kt{dt}")
                ps = ps_mm.tile([128, 1024], f32, tag="mm", name=f"psqk{dt}_{sb}")
                dsl = slice(128 * dt, 128 * (dt + 1))
                for ct in range(NCT):
                    nc.tensor.matmul(
                        ps[:, 0:512],
                        lhsT=wq[ct][:, dsl],
                        rhs=xc[ct][:, :],
                        start=(ct == 0),
                        stop=(ct == NCT - 1),
                    )
                for ct in range(NCT):
                    nc.tensor.matmul(
                        ps[:, 512:1024],
                        lhsT=wk[ct][:, dsl],
                        rhs=xc[ct][:, :],
                        start=(ct == 0),
                        stop=(ct == NCT - 1),
                    )
                ssl = slice(512 * sb, 512 * (sb + 1))
                nc.vector.tensor_copy(out=QT[dt][:, ssl], in_=ps[:, 0:512])
                nc.vector.tensor_copy(out=KT[dt][:, ssl], in_=ps[:, 512:1024])

            def v_chains(sb, xc):
                """V projection for the 4 s-tiles of s-block sb."""
                for half in range(2):
                    ps = ps_mm.tile([128, 1024], f32, tag="mm", name=f"psv{sb}_{half}")
                    for loc in range(2):
                        stl = 2 * half + loc
                        for ct in range(NCT):
                            nc.tensor.matmul(
                                ps[:, 512 * loc : 512 * (loc + 1)],
                                lhsT=xc[ct][:, 128 * stl : 128 * (stl + 1)],
                                rhs=wv[ct][:, :],
                                start=(ct == 0),
                                stop=(ct == NCT - 1),
                            )
                    for loc in range(2):
                        st = 4 * sb + 2 * half + loc
                        vt = p_v.tile([128, HL * 65], bf16, tag="v", name=f"v{st}")
                        V[st] = vt
                        # ones in column 64 of each head's 65-wide strip
                        nc.sync.dma_start(
                            out=vt[:, :].rearrange("p (h x) -> p h x", x=65)[:, :, 64:65],
                            in_=onesd[:, 0:HL],
                        )
                        nc.vector.tensor_copy(
                            out=vt[:, :].rearrange("p (h x) -> p h x", x=65)[:, :, 0:64],
                            in_=ps[:, 512 * loc : 512 * (loc + 1)].rearrange(
                                "p (h x) -> p h x", x=64
                            ),
                        )

            # --- phase 1: V + Q/K for d-tile 0 ---
            for sb in range(NSB):
                xc = load_xchunks(sb)
                if sb == 0:
                    load_wqk()
                v_chains(sb, xc)
                qk_chain(0, sb, xc)

            # deferred QK-projection tasks for later head pairs, interleaved
            # into the previous pair's attention to keep ACT fed
            def make_qk_tasks(dt):
                def task(sb=None, dt=dt):
                    xc = load_xchunks(sb)
                    qk_chain(dt, sb, xc)

                return [lambda sb=sb: task(sb) for sb in range(NSB)]

            pending = {j: make_qk_tasks(j) for j in range(1, NDT)}

            # deferred W_o chains (by q-block), interleaved into pair-3
            wo_tiles = {}

            def load_wo():
                for jj in range(NDT):
                    for eb in range(NEB):
                        t = p_w.tile([128, 512], bf16, tag="w", name=f"wo{jj}_{eb}")
                        nc.sync.dma_start(
                            out=t[:, :],
                            in_=woT[
                                128 * jj : 128 * (jj + 1), 512 * eb : 512 * (eb + 1)
                            ],
                        )
                        wo_tiles[(jj, eb)] = t

            wo_queue = []

            def wo_one(qt, eb):
                """One output-projection chain: out tile [128q, 512e]."""
                qsl = slice(128 * qt, 128 * (qt + 1))
                ps = ps_mm.tile([128, 512], f32, tag="mm", name=f"pso{qt}_{eb}")
                for jj in range(NDT):
                    nc.tensor.matmul(
                        ps[:, :],
                        lhsT=cat[jj][:, qsl],
                        rhs=wo_tiles[(jj, eb)][:, :],
                        start=(jj == 0),
                        stop=(jj == NDT - 1),
                    )
                ost = p_out.tile([128, 512], f32, tag="outst", name=f"o{qt}_{eb}")
                nc.vector.tensor_copy(out=ost[:, :], in_=ps[:, :])
                nc.sync.dma_start(
                    out=out[qsl, 512 * eb : 512 * (eb + 1)], in_=ost[:, :]
                )

            def wo_chains(qb):
                for qtl in range(4):
                    for eb in range(NEB):
                        wo_queue.append(
                            lambda qt=4 * qb + qtl, eb=eb: wo_one(qt, eb)
                        )

            # --- phase 2: attention per head pair, software-pipelined ---
            # Within a pair, scores+exp for q-block `it` run one iteration
            # ahead of the attn@V (aug) matmuls for `it-1`, and normalization
            # for `it-2` trails another iteration.  The PE therefore always
            # has ready work queued and never idles long enough for the HAM
            # clock gate to re-throttle it to 1.2 GHz.
            for j in range(NDT):
                cat[j] = p_cat.tile([128, S], bf16, tag="cat", name=f"cat{j}")
            load_wo()
            pts = {}
            augs = {}
            s2s = {}
            interleave = []

            def scores_exp(j, qb, g):
                qtj, ktj = QT[j], KT[j]
                qsl = slice(512 * qb, 512 * (qb + 1))
                psA = ps_mm.tile([128, 1024], f32, tag="mm", name=f"psA{j}{qb}{g}")
                psB = ps_mm.tile([128, 1024], f32, tag="mm", name=f"psB{j}{qb}{g}")
                for i in range(2):
                    kt = 2 * g + i
                    ksl = slice(128 * kt, 128 * (kt + 1))
                    osl = slice(512 * i, 512 * (i + 1))
                    nc.tensor.matmul(
                        psA[:, osl], lhsT=ktj[0:64, ksl], rhs=qtj[0:64, qsl],
                        start=True, stop=True,
                    )
                    nc.tensor.matmul(
                        psB[:, osl], lhsT=ktj[64:128, ksl], rhs=qtj[64:128, qsl],
                        start=True, stop=True,
                    )
                ptA = p_pt.tile([128, 1024], bf16, tag="pt", name=f"ptA{j}{qb}{g}")
                ptB = p_pt.tile([128, 1024], bf16, tag="pt", name=f"ptB{j}{qb}{g}")
                nc.scalar.activation(out=ptA[:, :], in_=psA[:, :], func=EXP, scale=0.125)
                nc.scalar.activation(out=ptB[:, :], in_=psB[:, :], func=EXP, scale=0.125)
                pts[(j, qb, g)] = (ptA, ptB)

            aug_ps = {}

            def aug_mms(j, qb, g):
                hA, hB = 2 * j, 2 * j + 1
                if g % 4 == 0:
                    # fresh PSUM accumulator pair per 4-group burst; partial
                    # results drain to SBUF so only 2 PSUM banks stay live
                    aug_ps[(j, qb)] = (
                        ps_aug.tile([65, 512], f32, tag="aug", name=f"psgA{j}{qb}{g}"),
                        ps_aug.tile([65, 512], f32, tag="aug", name=f"psgB{j}{qb}{g}"),
                    )
                pA, pB = aug_ps[(j, qb)]
                ptA, ptB = pts.pop((j, qb, g))
                for i in range(2):
                    kt = 2 * g + i
                    nc.tensor.matmul(
                        pA[:, :], lhsT=V[kt][:, 65 * hA : 65 * hA + 65],
                        rhs=ptA[:, 512 * i : 512 * (i + 1)],
                        start=(g % 4 == 0 and i == 0), stop=(g % 4 == 3 and i == 1),
                    )
                    nc.tensor.matmul(
                        pB[:, :], lhsT=V[kt][:, 65 * hB : 65 * hB + 65],
                        rhs=ptB[:, 512 * i : 512 * (i + 1)],
                        start=(g % 4 == 0 and i == 0), stop=(g % 4 == 3 and i == 1),
                    )
                if g % 4 == 3:
                    pA, pB = aug_ps.pop((j, qb))
                    if (j, qb) not in augs:
                        accA = p_acc.tile([65, 512], f32, tag="acc", name=f"accA{j}_{qb}")
                        accB = p_acc.tile([65, 512], f32, tag="acc", name=f"accB{j}_{qb}")
                        augs[(j, qb)] = (accA, accB)
                        nc.vector.tensor_copy(out=accA[:, :], in_=pA[:, :])
                        nc.vector.tensor_copy(out=accB[:, :], in_=pB[:, :])
                    else:
                        accA, accB = augs[(j, qb)]
                        nc.vector.tensor_add(out=accA[:, :], in0=accA[:, :], in1=pA[:, :])
                        nc.vector.tensor_add(out=accB[:, :], in0=accB[:, :], in1=pB[:, :])

            def norm_gather(j, qb):
                # pull the two softmax-denominator rows into SBUF (DVE),
                # then assemble them on adjacent partitions via tiny DMAs
                # (DVE cannot write to partition 1, DMA can)
                accA, accB = augs[(j, qb)]
                s2 = p_recip.tile([2, 512], f32r, tag="s2", name=f"s2_{j}_{qb}")
                nc.sync.dma_start(out=s2[0:1, :], in_=accA[64:65, :].bitcast(f32r))
                nc.sync.dma_start(out=s2[1:2, :], in_=accB[64:65, :].bitcast(f32r))
                s2s[(j, qb)] = s2

            def norm_apply(j, qb):
                # broadcast sums along partitions (PE outer product),
                # reciprocal on DVE, then scale into catT
                qsl = slice(512 * qb, 512 * (qb + 1))
                s2 = s2s.pop((j, qb))
                bc = ps_mm.tile([128, 512], f32, tag="mm", name=f"bc{j}_{qb}")
                nc.tensor.matmul(
                    bc[:, :], lhsT=ind2[:, :], rhs=s2[:, :], start=True, stop=True
                )
                rec = p_recip.tile([128, 512], f32, tag="rec", name=f"rec{j}_{qb}")
                nc.vector.reciprocal(out=rec[:, :], in_=bc[:, :])
                # SBUF+SBUF operands must share a base partition: re-base B's half
                recB = p_recip.tile([64, 512], f32, tag="recB", name=f"recB{j}_{qb}")
                nc.vector.tensor_copy(out=recB[:, :], in_=rec[64:128, :])
                accA, accB = augs.pop((j, qb))
                nc.vector.tensor_mul(
                    out=cat[j][0:64, qsl], in0=accA[0:64, :], in1=rec[0:64, :]
                )
                nc.vector.tensor_mul(
                    out=cat[j][64:128, qsl], in0=accB[0:64, :], in1=recB[:, :]
                )
                if j == NDT - 1:
                    wo_chains(qb)

            # one flat pipeline over all (pair, q-block) items: the aug
            # stream trails the scores stream by one item and crosses pair
            # boundaries without draining, so the PE stays dense end to end
            items = [(j, qb) for j in range(NDT) for qb in range(NQB)]
            NI = len(items)
            for idx in range(NI + 2):
                if idx < NI and items[idx][1] == 0 and items[idx][0] + 1 < NDT:
                    # queue projection chains for the next pair
                    interleave.extend(pending.pop(items[idx][0] + 1))
                for g in range(8):
                    if idx < NI:
                        scores_exp(*items[idx], g)
                    if 1 <= idx <= NI:
                        aug_mms(*items[idx - 1], g)
                    if 2 <= idx and g == 4 and items[idx - 2] in s2s:
                        norm_apply(*items[idx - 2])
                    if g % 2 == 1 and interleave:
                        interleave.pop(0)()
                    if wo_queue:
                        wo_queue.pop(0)()
                if 1 <= idx <= NI:
                    norm_gather(*items[idx - 1])
            for t in interleave:
                t()
            for t in wo_queue:
                t()

    _split_matmul_waits(nc)
    return nc


_SPLIT_TYPES = {"InstMatmult", "InstDMACopy", "InstActivation", "InstTensorCopy", "InstTensorTensor", "InstMemSet", "InstTensorScalarPtr", "InstTensorReduce", "InstReciprocal", "InstDrain", "InstNoOp", "InstEventSemaphore"}


def _split_matmul_waits(nc):
    """Several walrus instruction structs (fused-weight-load matmul S3_LW,
    DMA_DIRECT2D, ...) accept only one sync wait.  Move extra waits onto
    standalone no-ops on the same engine placed just before the instruction."""
    import concourse.mybir as mybir

    noop_cls = None
    for f in nc.m.functions:
        for blk in f.blocks:
            patched = []
            for inst in blk.instructions:
                si = getattr(inst, "sync_info", None)
                if (
                    type(inst).__name__ in _SPLIT_TYPES
                    and si is not None
                    and si.on_wait
                    and len(si.on_wait) > 1
                ):
                    if noop_cls is None:
                        import bass_rust

                        noop_cls = bass_rust.InstNoOp
                    waits = list(si.on_wait)
                    for w in waits[:-1]:
                        nop = noop_cls(
                            name=f"I-wsplit-{nc.next_id()}",
                            engine=inst.engine,
                            ins=[],
                            outs=[],
                        )
                        nop.sync_info = mybir.SyncInfo(on_wait=[w], on_update=[])
                        patched.append(nop)
                    inst.sync_info = mybir.SyncInfo(
                        on_wait=[waits[-1]], on_update=si.on_update
                    )
                patched.append(inst)
            blk.instructions[:] = patched


_CACHE = {}

import ml_dtypes

_BF16 = ml_dtypes.bfloat16

_IND2 = np.zeros((2, 128), dtype=np.float32)
_IND2[0, 0:64] = 1.0
_IND2[1, 64:128] = 1.0


def kernel(x, W_q, W_k, W_v, W_o):
    x = np.asarray(x, dtype=np.float32)
    W_q = np.asarray(W_q, dtype=np.float32)
    W_k = np.asarray(W_k, dtype=np.float32)
    W_v = np.asarray(W_v, dtype=np.float32)
    W_o = np.asarray(W_o, dtype=np.float32)

    if "nc" not in _CACHE:
        _CACHE["nc"] = build_bass()
    nc = _CACHE["nc"]

    from concourse.bass_utils import run_bass_kernel_spmd

    in_maps = []
    for c in range(8):
        b, j = divmod(c, 2)
        sl = slice(512 * j, 512 * (j + 1))
        in_maps.append(
            {
                "xT": np.ascontiguousarray(x[b].T).astype(_BF16),
                "wqT": np.ascontiguousarray(W_q[sl, :].T).astype(_BF16),
                "wkT": np.ascontiguousarray(W_k[sl, :].T).astype(_BF16),
                "wvT": np.ascontiguousarray(W_v[sl, :].T).astype(_BF16),
                "woT": np.ascontiguousarray(W_o[:, sl].T).astype(_BF16),
                "ones128": np.ones((128, 64), dtype=_BF16),
                "ind2": _IND2,
            }
        )

    res = run_bass_kernel_spmd(nc, in_maps, list(range(8))).results
    out = np.empty((4, S, C), dtype=np.float32)
    for b in range(4):
        out[b] = res[2 * b]["out"] + res[2 * b + 1]["out"]
    return out
